# revision 2
# baseline (speedup 1.0000x reference)
"""Fused RNN cell on 8 Trainium2 NeuronCores.

Reference computation (fp32):
    combined   = [x, hidden]                      [B=4096, I+H=4096]
    new_hidden = tanh(combined @ W_ih^T + b_ih)   [B, H=2048]
    output     = new_hidden @ W_ho^T + b_ho       [B, O=2048]
    returns (output, new_hidden)

Strategy: data-parallel over the batch — each of the 8 cores processes 512
batch rows with replicated weights; no collectives. All operand layout
transforms happen on the host so every device DMA is a fat transfer.

Matmuls run in fp8 e4m3 DoubleRow mode (0.5 PE cycles/row — 2x the fp16
rate; each instruction contracts 256 logical k as 128 partitions x 2
paired k-tiles). Direct e4m3 quantization alone would give ~3e-2 relative
error (> the 2e-2 gate), so both operands are split into hi + lo e4m3
residual streams and each matmul is computed as three DoubleRow matmuls

    acc = Wh@Ah + Wh@Al + Wl@Ah        (the lo@lo term is negligible)

accumulated in one fp32 PSUM bank. Measured end-to-end error ~2e-3. Net
PE cost: 1.5 DoubleRow MMs per fp16-equivalent MM = 0.75x fp16 PE time
(~123 us vs ~166 us), with identical HBM traffic (hi+lo fp8 == fp16
bytes). Weights are pre-scaled by 64 on the host so W*64 ~ N(0,1) sits in
e4m3's normal range; the 1/64 comes back for free via the ACT scale
operand at PSUM eviction.

Per 256-k step the three term-streams are ordered hh, hl, lh so the first
8 matmuls need only the hi-weight tile (sync ring) and the lo-weight tile
(scalar ring) gets an extra ~1.7 us of slack. hi weights + c_hi ride the
sync HWDGE ring; lo weights + c_lo ride the ACT ring (~15 MB each). nh
hi/lo fp8 splits for mm2's moving operand are computed on-chip by DVE
(copy + subtract) right behind each ACT tanh eviction; nhT stores ride
GpSimd SWDGE so neither HWDGE ring carries them. outT evictions alternate
DVE/ACT and their stores alternate sync/ACT rings, deferred one group so
a store waiting on compute never head-of-line blocks the load rings.
Dummy matmuls at t=0 warm the PE clock gate (HAM) and preload the ACT
tanh table during the initial DMA ramp.
"""

import numpy as np
import ml_dtypes

import concourse.bass as bass
import concourse.mybir as mybir
import concourse.tile as tile
from concourse import bacc, bass_utils

NCORES = 8
B, I, H, O = 4096, 2048, 2048, 2048
BC = B // NCORES          # 512 batch rows per core
K1 = I + H                # mm1 contraction dim, 4096
KO1 = K1 // 128           # 32 k-chunks for mm1
KP1 = KO1 // 2            # 16 k-pair steps for mm1
HC = H // 128             # 16 h-chunks
HP2 = HC // 2             # 8 k-pair steps for mm2
OC = O // 128             # 16 o-chunks
G = 8                     # h/o-chunks per PSUM group (8 banks)
P = 128
SW = 64.0                 # host weight pre-scale (into e4m3 normal range)
F32 = mybir.dt.float32
F16 = mybir.dt.float16
F8 = mybir.dt.float8e4
AF = mybir.ActivationFunctionType
DR = mybir.MatmulPerfMode.DoubleRow
E4 = ml_dtypes.float8_e4m3fn


def _build():
    nc = bacc.Bacc("TRN2", target_bir_lowering=False)

    ch = nc.dram_tensor("ch", [P, KO1, BC], F8, kind="ExternalInput")
    cl = nc.dram_tensor("cl", [P, KO1, BC], F8, kind="ExternalInput")
    w1h = nc.dram_tensor("w1h", [P, 2, KP1, 2, G, P], F8, kind="ExternalInput")
    w1l = nc.dram_tensor("w1l", [P, 2, KP1, 2, G, P], F8, kind="ExternalInput")
    b1 = nc.dram_tensor("b1", [P, HC], F32, kind="ExternalInput")
    w2h = nc.dram_tensor("w2h", [P, HP2, 2, OC, P], F8, kind="ExternalInput")
    w2l = nc.dram_tensor("w2l", [P, HP2, 2, OC, P], F8, kind="ExternalInput")
    nhT = nc.dram_tensor("nhT", [H, BC], F16, kind="ExternalOutput")
    outT = nc.dram_tensor("outT", [O, BC], F16, kind="ExternalOutput")

    with tile.TileContext(nc) as tc:
        with tc.tile_pool(name="cpool", bufs=1) as cpool, \
             tc.tile_pool(name="wpool", bufs=12) as wpool, \
             tc.tile_pool(name="nhpool", bufs=1) as nhpool, \
             tc.tile_pool(name="opool", bufs=8) as opool, \
             tc.tile_pool(name="bpool", bufs=1) as bpool, \
             tc.tile_pool(name="ps", bufs=8, space="PSUM") as ps:

            # PE warm-up: the HAM clock gate holds the PE at 1.2 GHz until
            # it has been busy ~3.4 µs. Dummy matmuls (no data deps beyond
            # one memset) keep the PE active while the first input tiles
            # stream in, so real matmuls start at 2.4 GHz.
            warm_sb = bpool.tile([P, P], F16)
            nc.gpsimd.memset(warm_sb[:], 0.0)

            b1_sb = bpool.tile([P, HC], F32)
            # b_ih isn't needed until the first group drains; keep it off
            # the HWDGE rings entirely (SWDGE via GpSimd).
            nc.gpsimd.dma_start(b1_sb[:], b1[:])

            ch_sb = cpool.tile([P, KO1, BC], F8)
            cl_sb = cpool.tile([P, KO1, BC], F8)
            nh16_sb = nhpool.tile([P, HC, BC], F16)
            nhh_sb = nhpool.tile([P, HC, BC], F8)
            nhl_sb = nhpool.tile([P, HC, BC], F8)

            # Stores are deferred one group: group g's stores are emitted
            # after group g+1's loads, so when the sync sequencer reaches
            # them the producing compute finished long ago and the ring
            # never head-of-line blocks on a store waiting for compute.
            deferred = []

            def flush_deferred():
                for fn in deferred:
                    fn()
                deferred.clear()

            # mm1: nh^T[h, b] = tanh((W_ih*64) @ combined^T / 64 + b_ih)
            # G-sized PSUM groups ping-pong across the 8 banks: while one
            # group's banks drain through ACT, the next group accumulates.
            for g in range(HC // G):
                psums = [ps.tile([P, BC], F32, tag="ps", name=f"ps{i}")
                         for i in range(G)]
                if g == 0:
                    # ~44 cold matmuls cover the ~3.4 us HAM window AND
                    # bridge until the first weight tile lands (~11.8 us)
                    # so no real matmul runs cold.
                    for _ in range(44):
                        nc.tensor.matmul(
                            psums[G - 1][:, :P], lhsT=warm_sb[:],
                            rhs=warm_sb[:],
                            start=True, stop=True, skip_group_check=True,
                        )
                for kp in range(KP1):
                    ko0 = 2 * kp
                    if g == 0 and kp % 2 == 0:
                        # c hi/lo stream 4 ko-chunks ahead of the weights
                        # on their respective rings.
                        nc.sync.dma_start(
                            ch_sb[:, ko0:ko0 + 4], ch[:, ko0:ko0 + 4])
                        nc.scalar.dma_start(
                            cl_sb[:, ko0:ko0 + 4], cl[:, ko0:ko0 + 4])
                        if kp == 2:
                            # Preload the ACT tanh table set (~1.3 us)
                            # during the ramp, not at the first drain.
                            act_warm = bpool.tile([1, 1], F32)
                            nc.scalar.activation(
                                act_warm[:], warm_sb[:1, :1], AF.Tanh)
                    w1h_sb = wpool.tile([P, 2, G, P], F8, tag="w", name="wh")
                    nc.sync.dma_start(w1h_sb[:], w1h[:, g, kp])
                    w1l_sb = wpool.tile([P, 2, G, P], F8, tag="w", name="wl")
                    nc.scalar.dma_start(w1l_sb[:], w1l[:, g, kp])
                    rhs_h = ch_sb[:, ko0:ko0 + 2]
                    rhs_l = cl_sb[:, ko0:ko0 + 2]
                    for i in range(G):
                        nc.tensor.matmul(
                            psums[i][:], lhsT=w1h_sb[:, :, i], rhs=rhs_h,
                            start=(kp == 0), stop=False, perf_mode=DR)
                    for i in range(G):
                        nc.tensor.matmul(
                            psums[i][:], lhsT=w1h_sb[:, :, i], rhs=rhs_l,
                            start=False, stop=False, perf_mode=DR)
                    for i in range(G):
                        nc.tensor.matmul(
                            psums[i][:], lhsT=w1l_sb[:, :, i], rhs=rhs_h,
                            start=False, stop=(kp == KP1 - 1), perf_mode=DR)
                flush_deferred()
                for i in range(G):
                    hc = g * G + i
                    nc.scalar.activation(
                        nh16_sb[:, hc], psums[i][:], AF.Tanh,
                        bias=b1_sb[:, hc:hc + 1], scale=1.0 / SW,
                    )
                    # mm2's moving operand: hi/lo e4m3 split, on DVE.
                    nc.vector.tensor_copy(nhh_sb[:, hc], nh16_sb[:, hc])
                    nc.vector.tensor_sub(
                        nhl_sb[:, hc], nh16_sb[:, hc], nhh_sb[:, hc])
                    # nhT stores ride SWDGE: no HWDGE ring time spent.
                    nc.gpsimd.dma_start(
                        nhT[hc * P:(hc + 1) * P, :], nh16_sb[:, hc])

            # mm2: out^T[o, b] = (W_ho*64) @ nh^T / 64   (b_ho added on host)
            # Groups of [8, 4, 2, 2] o-chunks: trailing groups ping-pong
            # through the 8 PSUM banks (no boundary stall) and shrink so
            # the post-last-matmul drain chain is as short as possible.
            for g0, gsz in ((0, 8), (8, 4), (12, 2), (14, 2)):
                psums = [ps.tile([P, BC], F32, tag="ps", name=f"ps{i}")
                         for i in range(gsz)]
                for hp in range(HP2):
                    ho0 = 2 * hp
                    w2h_sb = wpool.tile(
                        [P, 2, G, P], F8, tag="w", name="w2h")[:, :, :gsz]
                    nc.sync.dma_start(w2h_sb[:], w2h[:, hp, :, g0:g0 + gsz])
                    w2l_sb = wpool.tile(
                        [P, 2, G, P], F8, tag="w", name="w2l")[:, :, :gsz]
                    nc.scalar.dma_start(w2l_sb[:], w2l[:, hp, :, g0:g0 + gsz])
                    rhs_h = nhh_sb[:, ho0:ho0 + 2]
                    rhs_l = nhl_sb[:, ho0:ho0 + 2]
                    for i in range(gsz):
                        nc.tensor.matmul(
                            psums[i][:], lhsT=w2h_sb[:, :, i], rhs=rhs_h,
                            start=(hp == 0), stop=False, perf_mode=DR)
                    for i in range(gsz):
                        nc.tensor.matmul(
                            psums[i][:], lhsT=w2h_sb[:, :, i], rhs=rhs_l,
                            start=False, stop=False, perf_mode=DR)
                    for i in range(gsz):
                        nc.tensor.matmul(
                            psums[i][:], lhsT=w2l_sb[:, :, i], rhs=rhs_h,
                            start=False, stop=(hp == HP2 - 1), perf_mode=DR)
                flush_deferred()
                # Evict PSUM through both DVE and ACT in parallel (x 1/64
                # to undo the weight pre-scale). ACT-evicted tiles store
                # via the ACT HWDGE ring right behind their copy;
                # DVE-evicted tiles store via the sync ring, deferred one
                # group so the ring never waits on the copy.
                last = (g0 + gsz == OC)
                for i in range(gsz):
                    oc = g0 + i
                    o_sb = opool.tile([P, BC], F16, tag="osb")
                    if i % 2:
                        nc.scalar.activation(
                            o_sb[:], psums[i][:], AF.Copy, scale=1.0 / SW)
                        nc.scalar.dma_start(
                            outT[oc * P:(oc + 1) * P, :], o_sb[:])
                    else:
                        nc.vector.tensor_scalar_mul(
                            o_sb[:], psums[i][:], 1.0 / SW)
                        st = (lambda oc=oc, o_sb=o_sb: nc.sync.dma_start(
                            outT[oc * P:(oc + 1) * P, :], o_sb[:]))
                        if last:
                            st()      # no deferral on the final group
                        else:
                            deferred.append(st)
            flush_deferred()

    nc.compile()
    return nc


def _q8(x):
    """fp32 -> e4m3 with TRN clipping (values here never approach 240)."""
    return np.clip(x, -240.0, 240.0).astype(E4)


def _split8(x):
    hi = _q8(x)
    lo = _q8(x - hi.astype(np.float32))
    return hi, lo


def _shard_inputs(x, hidden, W_ih, b_ih, W_ho, b_ho):
    combined = np.concatenate([x, hidden], axis=1)  # [B, K1]

    w1h, w1l = _split8(W_ih.astype(np.float32) * SW)
    # [hc, h, kp, kk, ki] view of [H, K1] -> [ki, g, kp, kk, hcg, h]
    w1hL = np.ascontiguousarray(
        w1h.reshape(2, G, P, KP1, 2, P).transpose(5, 0, 3, 4, 1, 2))
    w1lL = np.ascontiguousarray(
        w1l.reshape(2, G, P, KP1, 2, P).transpose(5, 0, 3, 4, 1, 2))

    w2h, w2l = _split8(W_ho.astype(np.float32) * SW)
    # [oc, o, hp, kk, hi] view of [O, H] -> [hi, hp, kk, oc, o]
    w2hL = np.ascontiguousarray(
        w2h.reshape(OC, P, HP2, 2, P).transpose(4, 2, 3, 0, 1))
    w2lL = np.ascontiguousarray(
        w2l.reshape(OC, P, HP2, 2, P).transpose(4, 2, 3, 0, 1))

    b1L = np.ascontiguousarray(b_ih.reshape(HC, P).T.astype(np.float32))

    in_maps = []
    for cix in range(NCORES):
        cc = combined[cix * BC:(cix + 1) * BC].astype(np.float32)  # [BC, K1]
        cch, ccl = _split8(cc)
        cL = np.ascontiguousarray(
            cch.reshape(BC, KO1, P).transpose(2, 1, 0))
        clL = np.ascontiguousarray(
            ccl.reshape(BC, KO1, P).transpose(2, 1, 0))
        in_maps.append(
            {"ch": cL, "cl": clL, "w1h": w1hL, "w1l": w1lL, "b1": b1L,
             "w2h": w2hL, "w2l": w2lL}
        )
    return in_maps


def _run(in_maps, **kwargs):
    nc = _build()
    return bass_utils.run_bass_kernel_spmd(
        nc, in_maps, core_ids=list(range(NCORES)), **kwargs
    )


def kernel(x, hidden, W_ih, b_ih, W_ho, b_ho):
    x = np.asarray(x, dtype=np.float32)
    hidden = np.asarray(hidden, dtype=np.float32)
    W_ih = np.asarray(W_ih, dtype=np.float32)
    b_ih = np.asarray(b_ih, dtype=np.float32)
    W_ho = np.asarray(W_ho, dtype=np.float32)
    b_ho = np.asarray(b_ho, dtype=np.float32)

    in_maps = _shard_inputs(x, hidden, W_ih, b_ih, W_ho, b_ho)
    res = _run(in_maps)
    output = np.concatenate(
        [r["outT"].T.astype(np.float32) for r in res.results], axis=0) + b_ho
    new_hidden = np.concatenate(
        [r["nhT"].T for r in res.results], axis=0).astype(np.float32)
    return output, new_hidden


# revision 3
# speedup vs baseline: 1.4151x; 1.4151x over previous
"""Fused RNN cell on 8 Trainium2 NeuronCores.

Reference computation (fp32):
    combined   = [x, hidden]                      [B=4096, I+H=4096]
    new_hidden = tanh(combined @ W_ih^T + b_ih)   [B, H=2048]
    output     = new_hidden @ W_ho^T + b_ho       [B, O=2048]
    returns (output, new_hidden)

Strategy: data-parallel over the batch — each of the 8 cores processes 512
batch rows with replicated weights; no collectives. All operand layout
transforms (transposes into PE-friendly [K-partition, free] form) happen on
the host so every device DMA is a fat, fully contiguous transfer.

Matmuls run in fp16 (full-rate on the PE — 216 ns per 512-col MM, 1 moving
column/cycle at 2.4 GHz; fp8 DoubleRow was measured at the SAME 512 cycles
per 512-col instruction on this silicon, so an error-compensated fp8
scheme is 1.5x SLOWER than fp16 — don't go back there). PSUM accumulation
is fp32. mm1 produces nh^T [h, b] fp16 tiles in SBUF, which feed mm2
directly as the streaming operand; mm2 produces out^T [o, b] stored fp16.
Outputs are un-transposed and upcast on the host after the gather; b_ho is
added on the host.

The kernel is PE-bound with a gap-free MM stream (768 x 216 ns = 166 us);
total time = ~7.3 us fixed engine-barrier preamble + time-to-first-weight
+ 166 us + drain tail. v2 attacks time-to-first-weight: every per-kp
weight tile is split in half across BOTH HWDGE rings (sync gets h-chunks
0-3, ACT gets 4-7) so the first real matmul can start ~2.5 us after the
rings open instead of ~5.9; the c stream rides GpSimd SWDGE (except the
first 4 ko-chunks, which lead the two HWDGE rings) so the rings carry
nothing but weights during mm1 group 0. nhT stores also ride SWDGE.
outT evictions alternate DVE/ACT, their stores alternate sync/ACT rings,
deferred one group so a store waiting on compute never head-of-line
blocks a load ring. Dummy matmuls at t=0 warm the PE clock gate (HAM) and
preload the ACT tanh table during the initial DMA ramp.
"""

import numpy as np

import concourse.bass as bass
import concourse.mybir as mybir
import concourse.tile as tile
from concourse import bacc, bass_utils

NCORES = 8
B, I, H, O = 4096, 2048, 2048, 2048
BC = B // NCORES          # 512 batch rows per core
K1 = I + H                # mm1 contraction dim, 4096
KO1 = K1 // 128           # 32 k-chunks for mm1
HC = H // 128             # 16 h-chunks
OC = O // 128             # 16 o-chunks
G = 8                     # h/o-chunks per PSUM group (8 banks)
P = 128
NWARM = 28                # dummy PE warm-up matmuls
F32 = mybir.dt.float32
F16 = mybir.dt.float16
AF = mybir.ActivationFunctionType
NPF16 = np.float16


def _build():
    nc = bacc.Bacc("TRN2", target_bir_lowering=False)

    c = nc.dram_tensor("c", [P, KO1, BC], F16, kind="ExternalInput")
    w1 = nc.dram_tensor("w1", [P, KO1, HC, P], F16, kind="ExternalInput")
    b1 = nc.dram_tensor("b1", [P, HC], F32, kind="ExternalInput")
    w2 = nc.dram_tensor("w2", [P, HC, OC, P], F16, kind="ExternalInput")
    nhT = nc.dram_tensor("nhT", [H, BC], F16, kind="ExternalOutput")
    outT = nc.dram_tensor("outT", [O, BC], F16, kind="ExternalOutput")

    with tile.TileContext(nc) as tc:
        with tc.tile_pool(name="cpool", bufs=1) as cpool, \
             tc.tile_pool(name="wpool", bufs=10) as wpool, \
             tc.tile_pool(name="nhpool", bufs=1) as nhpool, \
             tc.tile_pool(name="opool", bufs=8) as opool, \
             tc.tile_pool(name="bpool", bufs=1) as bpool, \
             tc.tile_pool(name="ps", bufs=8, space="PSUM") as ps:

            # PE warm-up: the HAM clock gate holds the PE at 1.2 GHz until
            # it has been busy ~3.4 µs. Dummy matmuls (no data deps beyond
            # one memset) keep the PE active while the first input tiles
            # stream in, so real matmuls start near 2.4 GHz. The memset
            # rides GpSimd, whose queue opens right after the preamble.
            warm_sb = bpool.tile([P, P], F16)
            nc.gpsimd.memset(warm_sb[:], 0.0)

            c_sb = cpool.tile([P, KO1, BC], F16)
            # c beyond the first 4 ko-chunks rides GpSimd SWDGE so both
            # HWDGE rings carry nothing but weight tiles during group 0.
            # 4-ko slices (512 KiB) stay comfortably ahead of consumption
            # (one 2-ko step every ~3.5 µs).
            for ko0 in range(4, KO1, 4):
                nc.gpsimd.dma_start(
                    c_sb[:, ko0:ko0 + 4], c[:, ko0:ko0 + 4])

            b1_sb = bpool.tile([P, HC], F32)
            # b_ih isn't needed until the first group drains (~65 µs);
            # keep it off the HWDGE rings entirely (SWDGE via GpSimd).
            nc.gpsimd.dma_start(b1_sb[:], b1[:])

            nh_sb = nhpool.tile([P, HC, BC], F16)

            # Stores are deferred one group: group g's stores are emitted
            # after group g+1's loads, so when the sync sequencer reaches
            # them the producing compute finished long ago and the ring
            # never head-of-line blocks on a store waiting for compute.
            deferred = []

            def flush_deferred():
                for fn in deferred:
                    fn()
                deferred.clear()

            # mm1: nh^T[h, b] = tanh(W_ih @ combined^T + b_ih)
            # G-sized PSUM groups ping-pong across the 8 banks: while one
            # group's banks drain through ACT, the next group accumulates
            # — group boundaries cost the PE almost nothing.
            for g in range(HC // G):
                psums = [ps.tile([P, BC], F32, tag="ps", name=f"ps{i}")
                         for i in range(G)]
                if g == 0:
                    # The first c and w1 half-tiles land ~2.5 µs after the
                    # rings open; the warm matmuls bridge until then and
                    # start the ~3.4 µs HAM ramp.
                    for _ in range(NWARM):
                        nc.tensor.matmul(
                            psums[G - 1][:, :P], lhsT=warm_sb[:],
                            rhs=warm_sb[:],
                            start=True, stop=True, skip_group_check=True,
                        )
                for ko0 in range(0, KO1, 2):
                    if g == 0 and ko0 == 0:
                        # First two c chunks lead each HWDGE ring so the
                        # first matmuls have their moving operand before
                        # the SWDGE c stream spins up.
                        nc.sync.dma_start(c_sb[:, 0:2], c[:, 0:2])
                        nc.scalar.dma_start(c_sb[:, 2:4], c[:, 2:4])
                    if g == 0 and ko0 == 4:
                        # Preload the ACT tanh table set (~1.3 µs) during
                        # the ramp, not at the first drain.
                        act_warm = bpool.tile([1, 1], F32)
                        nc.scalar.activation(
                            act_warm[:], warm_sb[:1, :1], AF.Tanh)
                    # Each kp weight tile is split across both rings:
                    # sync carries h-chunks 0-3, ACT carries 4-7. Halves
                    # land in parallel, so a full tile takes ~1.8 µs of
                    # ring time instead of ~3.5.
                    w1_sb = wpool.tile([P, 2, G, P], F16, tag="w")
                    h0 = g * G
                    nc.sync.dma_start(
                        w1_sb[:, :, :4], w1[:, ko0:ko0 + 2, h0:h0 + 4])
                    nc.scalar.dma_start(
                        w1_sb[:, :, 4:], w1[:, ko0:ko0 + 2, h0 + 4:h0 + 8])
                    for kk in range(2):
                        for i in range(G):
                            nc.tensor.matmul(
                                psums[i][:],
                                lhsT=w1_sb[:, kk, i],
                                rhs=c_sb[:, ko0 + kk],
                                start=(ko0 + kk == 0),
                                stop=(ko0 + kk == KO1 - 1),
                            )
                flush_deferred()
                for i in range(G):
                    hc = g * G + i
                    nc.scalar.activation(
                        nh_sb[:, hc], psums[i][:], AF.Tanh,
                        bias=b1_sb[:, hc:hc + 1],
                    )
                    # nhT stores ride SWDGE: no HWDGE ring time spent.
                    nc.gpsimd.dma_start(
                        nhT[hc * P:(hc + 1) * P, :], nh_sb[:, hc])

            # mm2: out^T[o, b] = W_ho @ nh^T + b_ho
            # Groups of [8, 4, 2, 2] o-chunks: trailing groups ping-pong
            # through the 8 PSUM banks (no boundary stall) and shrink so
            # the post-last-matmul drain chain is as short as possible.
            for g0, gsz in ((0, 8), (8, 4), (12, 2), (14, 2)):
                psums = [ps.tile([P, BC], F32, tag="ps", name=f"ps{i}")
                         for i in range(gsz)]
                half = gsz // 2
                for ho0 in range(0, HC, 2):
                    w2_sb = wpool.tile(
                        [P, 2, G, P], F16, tag="w", name="w2_sb")[:, :, :gsz]
                    nc.sync.dma_start(
                        w2_sb[:, :, :half],
                        w2[:, ho0:ho0 + 2, g0:g0 + half])
                    nc.scalar.dma_start(
                        w2_sb[:, :, half:],
                        w2[:, ho0:ho0 + 2, g0 + half:g0 + gsz])
                    for kk in range(2):
                        for i in range(gsz):
                            nc.tensor.matmul(
                                psums[i][:],
                                lhsT=w2_sb[:, kk, i],
                                rhs=nh_sb[:, ho0 + kk],
                                start=(ho0 + kk == 0),
                                stop=(ho0 + kk == HC - 1),
                            )
                flush_deferred()
                # Evict PSUM through both DVE and ACT in parallel (raw
                # copies; b_ho is added on the host). ACT-evicted tiles
                # store via the ACT HWDGE ring right behind their copy;
                # DVE-evicted tiles store via the sync ring, deferred one
                # group so the ring never waits on the copy.
                last = (g0 + gsz == OC)
                for i in range(gsz):
                    oc = g0 + i
                    o_sb = opool.tile([P, BC], F16, tag="osb")
                    if i % 2:
                        nc.scalar.activation(o_sb[:], psums[i][:], AF.Copy)
                        nc.scalar.dma_start(
                            outT[oc * P:(oc + 1) * P, :], o_sb[:])
                    else:
                        nc.vector.tensor_copy(o_sb[:], psums[i][:])
                        st = (lambda oc=oc, o_sb=o_sb: nc.sync.dma_start(
                            outT[oc * P:(oc + 1) * P, :], o_sb[:]))
                        if last:
                            st()      # no deferral on the final group
                        else:
                            deferred.append(st)
            flush_deferred()

    nc.compile()
    return nc


def _shard_inputs(x, hidden, W_ih, b_ih, W_ho, b_ho):
    combined = np.concatenate([x, hidden], axis=1)  # [B, K1]
    w1L = np.ascontiguousarray(
        W_ih.reshape(HC, P, KO1, P).transpose(3, 2, 0, 1)
    ).astype(NPF16)  # [ki, ko, hc, h]
    w2L = np.ascontiguousarray(
        W_ho.reshape(OC, P, HC, P).transpose(3, 2, 0, 1)
    ).astype(NPF16)  # [hi, ho, oc, o]
    b1L = np.ascontiguousarray(b_ih.reshape(HC, P).T)
    in_maps = []
    for cix in range(NCORES):
        cc = combined[cix * BC:(cix + 1) * BC]  # [BC, K1]
        cL = np.ascontiguousarray(
            cc.reshape(BC, KO1, P).transpose(2, 1, 0)).astype(NPF16)
        in_maps.append(
            {"c": cL, "w1": w1L, "b1": b1L, "w2": w2L}
        )
    return in_maps


def _run(in_maps, **kwargs):
    nc = _build()
    return bass_utils.run_bass_kernel_spmd(
        nc, in_maps, core_ids=list(range(NCORES)), **kwargs
    )


def kernel(x, hidden, W_ih, b_ih, W_ho, b_ho):
    x = np.asarray(x, dtype=np.float32)
    hidden = np.asarray(hidden, dtype=np.float32)
    W_ih = np.asarray(W_ih, dtype=np.float32)
    b_ih = np.asarray(b_ih, dtype=np.float32)
    W_ho = np.asarray(W_ho, dtype=np.float32)
    b_ho = np.asarray(b_ho, dtype=np.float32)

    in_maps = _shard_inputs(x, hidden, W_ih, b_ih, W_ho, b_ho)
    res = _run(in_maps)
    output = np.concatenate(
        [r["outT"].T.astype(np.float32) for r in res.results], axis=0) + b_ho
    new_hidden = np.concatenate(
        [r["nhT"].T for r in res.results], axis=0).astype(np.float32)
    return output, new_hidden


# revision 7
# speedup vs baseline: 1.4664x; 1.0363x over previous
"""Fused RNN cell on 8 Trainium2 NeuronCores.

Reference computation (fp32):
    combined   = [x, hidden]                      [B=4096, I+H=4096]
    new_hidden = tanh(combined @ W_ih^T + b_ih)   [B, H=2048]
    output     = new_hidden @ W_ho^T + b_ho       [B, O=2048]
    returns (output, new_hidden)

Strategy: data-parallel over the batch — each of the 8 cores processes 512
batch rows with replicated weights; no collectives. All operand layout
transforms (transposes into PE-friendly [K-partition, free] form) happen on
the host so every device DMA is a fat, fully contiguous transfer.

Matmuls run in fp16 (full-rate on the PE — 216 ns per 512-col MM, 1 moving
column/cycle at 2.4 GHz; fp8 DoubleRow was measured at the SAME 512 cycles
per 512-col instruction on this silicon, so an error-compensated fp8
scheme is 1.5x SLOWER than fp16 — don't go back there). PSUM accumulation
is fp32. mm1 produces nh^T [h, b] fp16 tiles in SBUF, which feed mm2
directly as the streaming operand; mm2 produces out^T [o, b] stored fp16.
Outputs are un-transposed and upcast on the host after the gather; b_ho is
added on the host.

The kernel is PE-bound with a gap-free MM stream (768 x 216 ns = 166 us);
total time = ~7.3 us fixed engine-barrier preamble + time-to-first-weight
+ 166 us + drain tail. v2 attacks time-to-first-weight: every per-kp
weight tile is split in half across BOTH HWDGE rings (sync gets h-chunks
0-3, ACT gets 4-7) so the first real matmul can start ~2.5 us after the
rings open instead of ~5.9; the c stream rides GpSimd SWDGE (except the
first 4 ko-chunks, which lead the two HWDGE rings) so the rings carry
nothing but weights during mm1 group 0. nhT stores also ride SWDGE.
outT evictions alternate DVE/ACT, their stores alternate sync/ACT rings,
deferred one group so a store waiting on compute never head-of-line
blocks a load ring. Dummy matmuls at t=0 warm the PE clock gate (HAM) and
preload the ACT tanh table during the initial DMA ramp.
"""

import numpy as np

import concourse.bass as bass
import concourse.mybir as mybir
import concourse.tile as tile
from concourse import bacc, bass_utils

NCORES = 8
B, I, H, O = 4096, 2048, 2048, 2048
BC = B // NCORES          # 512 batch rows per core
K1 = I + H                # mm1 contraction dim, 4096
KO1 = K1 // 128           # 32 k-chunks for mm1
HC = H // 128             # 16 h-chunks
OC = O // 128             # 16 o-chunks
G = 8                     # h/o-chunks per PSUM group (8 banks)
P = 128
NWARM = 30                # dummy PE warm-up matmuls
F32 = mybir.dt.float32
F16 = mybir.dt.float16
AF = mybir.ActivationFunctionType
NPF16 = np.float16


def _build():
    nc = bacc.Bacc("TRN2", target_bir_lowering=False)

    c = nc.dram_tensor("c", [P, KO1, BC], F16, kind="ExternalInput")
    w1 = nc.dram_tensor("w1", [P, KO1, HC, P], F16, kind="ExternalInput")
    b1 = nc.dram_tensor("b1", [P, HC], F32, kind="ExternalInput")
    w2 = nc.dram_tensor("w2", [P, HC, OC, P], F16, kind="ExternalInput")
    nhT = nc.dram_tensor("nhT", [H, BC], F16, kind="ExternalOutput")
    outT = nc.dram_tensor("outT", [O, BC], F16, kind="ExternalOutput")

    with tile.TileContext(nc) as tc:
        with tc.tile_pool(name="cpool", bufs=1) as cpool, \
             tc.tile_pool(name="wpool", bufs=10) as wpool, \
             tc.tile_pool(name="nhpool", bufs=1) as nhpool, \
             tc.tile_pool(name="opool", bufs=8) as opool, \
             tc.tile_pool(name="bpool", bufs=1) as bpool, \
             tc.tile_pool(name="ps", bufs=8, space="PSUM") as ps:

            # PE warm-up: the HAM clock gate holds the PE at 1.2 GHz until
            # it has been busy ~3.4 µs. Dummy matmuls (no data deps beyond
            # one memset) keep the PE active while the first input tiles
            # stream in, so real matmuls start near 2.4 GHz. The memset
            # rides GpSimd, whose queue opens right after the preamble.
            warm_sb = bpool.tile([P, P], F16)
            nc.gpsimd.memset(warm_sb[:], 0.0)

            b1_sb = bpool.tile([P, HC], F32)
            # b_ih isn't needed until the first group drains (~65 µs);
            # keep it off the HWDGE rings entirely (SWDGE via GpSimd).
            nc.gpsimd.dma_start(b1_sb[:], b1[:])

            c_sb = cpool.tile([P, KO1, BC], F16)
            nh_sb = nhpool.tile([P, HC, BC], F16)

            # Stores are deferred one group: group g's stores are emitted
            # after group g+1's loads, so when the sync sequencer reaches
            # them the producing compute finished long ago and the ring
            # never head-of-line blocks on a store waiting for compute.
            deferred = []

            def flush_deferred():
                for fn in deferred:
                    fn()
                deferred.clear()

            # mm1: nh^T[h, b] = tanh(W_ih @ combined^T + b_ih)
            # G-sized PSUM groups ping-pong across the 8 banks: while one
            # group's banks drain through ACT, the next group accumulates
            # — group boundaries cost the PE almost nothing.
            for g in range(HC // G):
                psums = [ps.tile([P, BC], F32, tag="ps", name=f"ps{i}")
                         for i in range(G)]
                if g == 0:
                    # The first c and w1 half-tiles land ~2.5 µs after the
                    # rings open; the warm matmuls bridge until then and
                    # start the ~3.4 µs HAM ramp.
                    for _ in range(NWARM):
                        nc.tensor.matmul(
                            psums[G - 1][:, :P], lhsT=warm_sb[:],
                            rhs=warm_sb[:],
                            start=True, stop=True, skip_group_check=True,
                        )
                for ko0 in range(0, KO1, 2):
                    if g == 0:
                        # c rides the ACT HWDGE ring: descriptor pushes for
                        # the first c and w1 tiles then run in parallel on
                        # two queues, and during all of group 0 the sync
                        # ring carries only weights.
                        nc.scalar.dma_start(c_sb[:, ko0:ko0 + 2], c[:, ko0:ko0 + 2])
                        if ko0 == 2:
                            # Preload the ACT tanh table set (~1.3 µs)
                            # during the ramp, not at the first drain.
                            act_warm = bpool.tile([1, 1], F32)
                            nc.scalar.activation(
                                act_warm[:], warm_sb[:1, :1], AF.Tanh)
                    w1_sb = wpool.tile([P, 2, G, P], F16, tag="w")
                    h0 = g * G
                    if g == 0 and ko0 <= 2:
                        # The first two weight tiles are split into two
                        # half pushes so the first matmuls can start after
                        # ~256 KiB instead of ~512 KiB of ring traffic.
                        nc.sync.dma_start(
                            w1_sb[:, :, :4], w1[:, ko0:ko0 + 2, h0:h0 + 4])
                        nc.sync.dma_start(
                            w1_sb[:, :, 4:], w1[:, ko0:ko0 + 2, h0 + 4:h0 + 8])
                    else:
                        nc.sync.dma_start(
                            w1_sb[:], w1[:, ko0:ko0 + 2, h0:h0 + G])
                    for kk in range(2):
                        for i in range(G):
                            nc.tensor.matmul(
                                psums[i][:],
                                lhsT=w1_sb[:, kk, i],
                                rhs=c_sb[:, ko0 + kk],
                                start=(ko0 + kk == 0),
                                stop=(ko0 + kk == KO1 - 1),
                            )
                flush_deferred()
                for i in range(G):
                    hc = g * G + i
                    nc.scalar.activation(
                        nh_sb[:, hc], psums[i][:], AF.Tanh,
                        bias=b1_sb[:, hc:hc + 1],
                    )
                    # nhT stores ride SWDGE: no HWDGE ring time spent.
                    nc.gpsimd.dma_start(
                        nhT[hc * P:(hc + 1) * P, :], nh_sb[:, hc])

            # mm2: out^T[o, b] = W_ho @ nh^T + b_ho
            # Groups of [8, 4, 2, 2] o-chunks: trailing groups ping-pong
            # through the 8 PSUM banks (no boundary stall) and shrink so
            # the post-last-matmul drain chain is as short as possible.
            for g0, gsz in ((0, 8), (8, 4), (12, 2), (14, 2)):
                psums = [ps.tile([P, BC], F32, tag="ps", name=f"ps{i}")
                         for i in range(gsz)]
                for ho0 in range(0, HC, 2):
                    w2_sb = wpool.tile(
                        [P, 2, G, P], F16, tag="w", name="w2_sb")[:, :, :gsz]
                    nc.sync.dma_start(
                        w2_sb[:], w2[:, ho0:ho0 + 2, g0:g0 + gsz])
                    for kk in range(2):
                        for i in range(gsz):
                            nc.tensor.matmul(
                                psums[i][:],
                                lhsT=w2_sb[:, kk, i],
                                rhs=nh_sb[:, ho0 + kk],
                                start=(ho0 + kk == 0),
                                stop=(ho0 + kk == HC - 1),
                            )
                flush_deferred()
                # Evict PSUM through both DVE and ACT in parallel (raw
                # copies; b_ho is added on the host). ACT-evicted tiles
                # store via the ACT HWDGE ring right behind their copy;
                # DVE-evicted tiles store via the sync ring, deferred one
                # group so the ring never waits on the copy.
                last = (g0 + gsz == OC)
                for i in range(gsz):
                    oc = g0 + i
                    o_sb = opool.tile([P, BC], F16, tag="osb")
                    if i % 2:
                        nc.scalar.activation(o_sb[:], psums[i][:], AF.Copy)
                        nc.scalar.dma_start(
                            outT[oc * P:(oc + 1) * P, :], o_sb[:])
                    else:
                        nc.vector.tensor_copy(o_sb[:], psums[i][:])
                        st = (lambda oc=oc, o_sb=o_sb: nc.sync.dma_start(
                            outT[oc * P:(oc + 1) * P, :], o_sb[:]))
                        if last:
                            st()      # no deferral on the final group
                        else:
                            deferred.append(st)
            flush_deferred()

    nc.compile()
    return nc


def _shard_inputs(x, hidden, W_ih, b_ih, W_ho, b_ho):
    combined = np.concatenate([x, hidden], axis=1)  # [B, K1]
    w1L = np.ascontiguousarray(
        W_ih.reshape(HC, P, KO1, P).transpose(3, 2, 0, 1)
    ).astype(NPF16)  # [ki, ko, hc, h]
    w2L = np.ascontiguousarray(
        W_ho.reshape(OC, P, HC, P).transpose(3, 2, 0, 1)
    ).astype(NPF16)  # [hi, ho, oc, o]
    b1L = np.ascontiguousarray(b_ih.reshape(HC, P).T)
    in_maps = []
    for cix in range(NCORES):
        cc = combined[cix * BC:(cix + 1) * BC]  # [BC, K1]
        cL = np.ascontiguousarray(
            cc.reshape(BC, KO1, P).transpose(2, 1, 0)).astype(NPF16)
        in_maps.append(
            {"c": cL, "w1": w1L, "b1": b1L, "w2": w2L}
        )
    return in_maps


def _run(in_maps, **kwargs):
    nc = _build()
    return bass_utils.run_bass_kernel_spmd(
        nc, in_maps, core_ids=list(range(NCORES)), **kwargs
    )


def kernel(x, hidden, W_ih, b_ih, W_ho, b_ho):
    x = np.asarray(x, dtype=np.float32)
    hidden = np.asarray(hidden, dtype=np.float32)
    W_ih = np.asarray(W_ih, dtype=np.float32)
    b_ih = np.asarray(b_ih, dtype=np.float32)
    W_ho = np.asarray(W_ho, dtype=np.float32)
    b_ho = np.asarray(b_ho, dtype=np.float32)

    in_maps = _shard_inputs(x, hidden, W_ih, b_ih, W_ho, b_ho)
    res = _run(in_maps)
    output = np.concatenate(
        [r["outT"].T.astype(np.float32) for r in res.results], axis=0) + b_ho
    new_hidden = np.concatenate(
        [r["nhT"].T for r in res.results], axis=0).astype(np.float32)
    return output, new_hidden


# revision 13
# speedup vs baseline: 1.5105x; 1.0301x over previous
"""Fused RNN cell on 8 Trainium2 NeuronCores.

Reference computation (fp32):
    combined   = [x, hidden]                      [B=4096, I+H=4096]
    new_hidden = tanh(combined @ W_ih^T + b_ih)   [B, H=2048]
    output     = new_hidden @ W_ho^T + b_ho       [B, O=2048]
    returns (output, new_hidden)

Strategy: data-parallel over the batch — each of the 8 cores processes 512
batch rows with replicated weights; no collectives. All operand layout
transforms (transposes into PE-friendly [K-partition, free] form) happen on
the host so every device DMA is a fat, fully contiguous transfer.

Matmuls run in fp16 (full-rate on the PE — 216 ns per 512-col MM, 1 moving
column/cycle at 2.4 GHz; fp8 DoubleRow was measured at the SAME 512 cycles
per 512-col instruction on this silicon, so an error-compensated fp8
scheme is 1.5x SLOWER than fp16 — don't go back there). PSUM accumulation
is fp32. mm1 produces nh^T [h, b] fp16 tiles in SBUF, which feed mm2
directly as the streaming operand; mm2 produces out^T [o, b] stored fp16.
Outputs are un-transposed and upcast on the host after the gather; b_ho is
added on the host.

The kernel is PE-bound with a gap-free MM stream (768 x 216 ns = 166 us);
total time = ~7.3 us fixed engine-barrier preamble + time-to-first-weight
+ 166 us + drain tail. v2 attacks time-to-first-weight: every per-kp
weight tile is split in half across BOTH HWDGE rings (sync gets h-chunks
0-3, ACT gets 4-7) so the first real matmul can start ~2.5 us after the
rings open instead of ~5.9; the c stream rides GpSimd SWDGE (except the
first 4 ko-chunks, which lead the two HWDGE rings) so the rings carry
nothing but weights during mm1 group 0. nhT stores also ride SWDGE.
outT evictions alternate DVE/ACT, their stores alternate sync/ACT rings,
deferred one group so a store waiting on compute never head-of-line
blocks a load ring. Dummy matmuls at t=0 warm the PE clock gate (HAM) and
preload the ACT tanh table during the initial DMA ramp.
"""

import numpy as np
import ml_dtypes

import concourse.bass as bass
import concourse.mybir as mybir
import concourse.tile as tile
from concourse import bacc, bass_utils

NCORES = 8
B, I, H, O = 4096, 2048, 2048, 2048
BC = B // NCORES          # 512 batch rows per core
K1 = I + H                # mm1 contraction dim, 4096
KO1 = K1 // 128           # 32 k-chunks for mm1
KP8 = 2                   # mm1 k-pair steps (256 k each) run in fp8 DoubleRow
KO16 = KO1 - 2 * KP8      # leading k-chunks run in fp16 (28)
HC = H // 128             # 16 h-chunks
OC = O // 128             # 16 o-chunks
G = 8                     # h/o-chunks per PSUM group (8 banks)
P = 128
SW = 64.0                 # mm1 weight pre-scale (shared by fp16 and fp8 parts)
NWARM = 30                # dummy PE warm-up matmuls
F32 = mybir.dt.float32
F16 = mybir.dt.float16
F8 = mybir.dt.float8e4
AF = mybir.ActivationFunctionType
DR = mybir.MatmulPerfMode.DoubleRow
E4 = ml_dtypes.float8_e4m3fn
NPF16 = np.float16


def _build():
    nc = bacc.Bacc("TRN2", target_bir_lowering=False)

    c = nc.dram_tensor("c", [P, KO16, BC], F16, kind="ExternalInput")
    c8 = nc.dram_tensor("c8", [P, 2 * KP8, BC], F8, kind="ExternalInput")
    w1 = nc.dram_tensor("w1", [P, KO16, HC, P], F16, kind="ExternalInput")
    w18 = nc.dram_tensor("w18", [P, KP8, 2, HC, P], F8, kind="ExternalInput")
    b1 = nc.dram_tensor("b1", [P, HC], F32, kind="ExternalInput")
    w2 = nc.dram_tensor("w2", [P, HC, OC, P], F16, kind="ExternalInput")
    nhT = nc.dram_tensor("nhT", [H, BC], F16, kind="ExternalOutput")
    outT = nc.dram_tensor("outT", [O, BC], F16, kind="ExternalOutput")

    with tile.TileContext(nc) as tc:
        with tc.tile_pool(name="cpool", bufs=1) as cpool, \
             tc.tile_pool(name="wpool", bufs=10) as wpool, \
             tc.tile_pool(name="nhpool", bufs=1) as nhpool, \
             tc.tile_pool(name="opool", bufs=8) as opool, \
             tc.tile_pool(name="bpool", bufs=1) as bpool, \
             tc.tile_pool(name="ps", bufs=8, space="PSUM") as ps:

            # PE warm-up: the HAM clock gate holds the PE at 1.2 GHz until
            # it has been busy ~3.4 µs. Dummy matmuls (no data deps beyond
            # one memset) keep the PE active while the first input tiles
            # stream in, so real matmuls start near 2.4 GHz. The memset
            # rides GpSimd, whose queue opens right after the preamble.
            warm_sb = bpool.tile([P, P], F16)
            nc.gpsimd.memset(warm_sb[:], 0.0)

            b1_sb = bpool.tile([P, HC], F32)
            # b_ih isn't needed until the first group drains (~65 µs);
            # keep it off the HWDGE rings entirely (SWDGE via GpSimd).
            nc.gpsimd.dma_start(b1_sb[:], b1[:])

            c_sb = cpool.tile([P, KO16, BC], F16)
            c8_sb = cpool.tile([P, 2 * KP8, BC], F8)
            nh_sb = nhpool.tile([P, HC, BC], F16)

            # Stores are deferred one group: group g's stores are emitted
            # after group g+1's loads, so when the sync sequencer reaches
            # them the producing compute finished long ago and the ring
            # never head-of-line blocks on a store waiting for compute.
            deferred = []

            def flush_deferred():
                for fn in deferred:
                    fn()
                deferred.clear()

            # mm1: nh^T[h, b] = tanh(W_ih @ combined^T + b_ih)
            # G-sized PSUM groups ping-pong across the 8 banks: while one
            # group's banks drain through ACT, the next group accumulates
            # — group boundaries cost the PE almost nothing.
            for g in range(HC // G):
                psums = [ps.tile([P, BC], F32, tag="ps", name=f"ps{i}")
                         for i in range(G)]
                if g == 0:
                    # The first c and w1 half-tiles land ~2.5 µs after the
                    # rings open; the warm matmuls bridge until then and
                    # start the ~3.4 µs HAM ramp.
                    for _ in range(NWARM):
                        nc.tensor.matmul(
                            psums[G - 1][:, :P], lhsT=warm_sb[:],
                            rhs=warm_sb[:],
                            start=True, stop=True, skip_group_check=True,
                        )
                h0 = g * G
                for ko0 in range(0, KO16, 2):
                    if g == 0:
                        # c rides the ACT HWDGE ring: descriptor pushes for
                        # the first c and w1 tiles then run in parallel on
                        # two queues, and during all of group 0 the sync
                        # ring carries only weights.
                        nc.scalar.dma_start(c_sb[:, ko0:ko0 + 2], c[:, ko0:ko0 + 2])
                        if ko0 == 2:
                            # Preload the ACT tanh table set (~1.3 µs)
                            # during the ramp, not at the first drain.
                            act_warm = bpool.tile([1, 1], F32)
                            nc.scalar.activation(
                                act_warm[:], warm_sb[:1, :1], AF.Tanh)
                        if ko0 == 4:
                            # fp8 tail of c: tiny (512 KiB/8), needed only
                            # at the end of the group — push during ramp.
                            nc.scalar.dma_start(c8_sb[:], c8[:])
                    w1_sb = wpool.tile([P, 2, G, P], F16, tag="w")
                    if g == 0 and ko0 <= 2:
                        # The first two weight tiles are split into two
                        # half pushes so the first matmuls can start after
                        # ~256 KiB instead of ~512 KiB of ring traffic.
                        nc.sync.dma_start(
                            w1_sb[:, :, :4], w1[:, ko0:ko0 + 2, h0:h0 + 4])
                        nc.sync.dma_start(
                            w1_sb[:, :, 4:], w1[:, ko0:ko0 + 2, h0 + 4:h0 + 8])
                    else:
                        nc.sync.dma_start(
                            w1_sb[:], w1[:, ko0:ko0 + 2, h0:h0 + G])
                    for kk in range(2):
                        for i in range(G):
                            nc.tensor.matmul(
                                psums[i][:],
                                lhsT=w1_sb[:, kk, i],
                                rhs=c_sb[:, ko0 + kk],
                                start=(ko0 + kk == 0),
                                stop=False,
                            )
                # fp8 DoubleRow tail: each instruction contracts 256 k
                # (2 paired k-tiles) at the same 512-cycle cost as one
                # fp16 matmul — 2x FLOPs/instruction. Both the fp16 and
                # fp8 partials carry the same x64 weight pre-scale, so
                # they accumulate into the SAME PSUM bank; ACT's
                # scale=1/64 undoes it at eviction.
                for kp in range(KP8):
                    w18_sb = wpool.tile([P, 2, G, P], F8, tag="w", name="w18")
                    nc.sync.dma_start(w18_sb[:], w18[:, kp, :, h0:h0 + G])
                    for i in range(G):
                        nc.tensor.matmul(
                            psums[i][:],
                            lhsT=w18_sb[:, :, i],
                            rhs=c8_sb[:, 2 * kp:2 * kp + 2],
                            start=False,
                            stop=(kp == KP8 - 1),
                            perf_mode=DR,
                        )
                flush_deferred()
                for i in range(G):
                    hc = g * G + i
                    nc.scalar.activation(
                        nh_sb[:, hc], psums[i][:], AF.Tanh,
                        bias=b1_sb[:, hc:hc + 1], scale=1.0 / SW,
                    )
                    # nhT stores ride SWDGE: no HWDGE ring time spent.
                    nc.gpsimd.dma_start(
                        nhT[hc * P:(hc + 1) * P, :], nh_sb[:, hc])

            # mm2: out^T[o, b] = W_ho @ nh^T + b_ho
            # Groups of [8, 4, 2, 2] o-chunks: trailing groups ping-pong
            # through the 8 PSUM banks (no boundary stall) and shrink so
            # the post-last-matmul drain chain is as short as possible.
            for g0, gsz in ((0, 8), (8, 4), (12, 2), (14, 2)):
                psums = [ps.tile([P, BC], F32, tag="ps", name=f"ps{i}")
                         for i in range(gsz)]
                for ho0 in range(0, HC, 2):
                    w2_sb = wpool.tile(
                        [P, 2, G, P], F16, tag="w", name="w2_sb")[:, :, :gsz]
                    nc.sync.dma_start(
                        w2_sb[:], w2[:, ho0:ho0 + 2, g0:g0 + gsz])
                    for kk in range(2):
                        for i in range(gsz):
                            nc.tensor.matmul(
                                psums[i][:],
                                lhsT=w2_sb[:, kk, i],
                                rhs=nh_sb[:, ho0 + kk],
                                start=(ho0 + kk == 0),
                                stop=(ho0 + kk == HC - 1),
                            )
                flush_deferred()
                # Evict PSUM through both DVE and ACT in parallel (raw
                # copies; b_ho is added on the host). ACT-evicted tiles
                # store via the ACT HWDGE ring right behind their copy;
                # DVE-evicted tiles store via the sync ring, deferred one
                # group so the ring never waits on the copy.
                last = (g0 + gsz == OC)
                for i in range(gsz):
                    oc = g0 + i
                    o_sb = opool.tile([P, BC], F16, tag="osb")
                    if i % 2:
                        nc.scalar.activation(o_sb[:], psums[i][:], AF.Copy)
                        nc.scalar.dma_start(
                            outT[oc * P:(oc + 1) * P, :], o_sb[:])
                    else:
                        nc.vector.tensor_copy(o_sb[:], psums[i][:])
                        st = (lambda oc=oc, o_sb=o_sb: nc.sync.dma_start(
                            outT[oc * P:(oc + 1) * P, :], o_sb[:]))
                        if last:
                            st()      # no deferral on the final group
                        else:
                            deferred.append(st)
            flush_deferred()

    nc.compile()
    return nc


def _shard_inputs(x, hidden, W_ih, b_ih, W_ho, b_ho):
    combined = np.concatenate([x, hidden], axis=1)  # [B, K1]
    K16 = KO16 * P                                  # fp16 k-range (3584)
    W1s = W_ih.astype(np.float32) * SW
    w1L = np.ascontiguousarray(
        W1s[:, :K16].reshape(HC, P, KO16, P).transpose(3, 2, 0, 1)
    ).astype(NPF16)  # [ki, ko, hc, h]
    w18L = np.ascontiguousarray(
        np.clip(W1s[:, K16:], -240, 240).astype(E4)
        .reshape(HC, P, KP8, 2, P).transpose(4, 2, 3, 0, 1)
    )  # [ki, kp, kk, hc, h]
    w2L = np.ascontiguousarray(
        W_ho.reshape(OC, P, HC, P).transpose(3, 2, 0, 1)
    ).astype(NPF16)  # [hi, ho, oc, o]
    b1L = np.ascontiguousarray(b_ih.reshape(HC, P).T)
    in_maps = []
    for cix in range(NCORES):
        cc = combined[cix * BC:(cix + 1) * BC]  # [BC, K1]
        cL = np.ascontiguousarray(
            cc[:, :K16].reshape(BC, KO16, P).transpose(2, 1, 0)).astype(NPF16)
        c8L = np.ascontiguousarray(
            np.clip(cc[:, K16:], -240, 240).astype(E4)
            .reshape(BC, 2 * KP8, P).transpose(2, 1, 0))
        in_maps.append(
            {"c": cL, "c8": c8L, "w1": w1L, "w18": w18L,
             "b1": b1L, "w2": w2L}
        )
    return in_maps


def _run(in_maps, **kwargs):
    nc = _build()
    return bass_utils.run_bass_kernel_spmd(
        nc, in_maps, core_ids=list(range(NCORES)), **kwargs
    )


def kernel(x, hidden, W_ih, b_ih, W_ho, b_ho):
    x = np.asarray(x, dtype=np.float32)
    hidden = np.asarray(hidden, dtype=np.float32)
    W_ih = np.asarray(W_ih, dtype=np.float32)
    b_ih = np.asarray(b_ih, dtype=np.float32)
    W_ho = np.asarray(W_ho, dtype=np.float32)
    b_ho = np.asarray(b_ho, dtype=np.float32)

    in_maps = _shard_inputs(x, hidden, W_ih, b_ih, W_ho, b_ho)
    res = _run(in_maps)
    output = np.concatenate(
        [r["outT"].T.astype(np.float32) for r in res.results], axis=0) + b_ho
    new_hidden = np.concatenate(
        [r["nhT"].T for r in res.results], axis=0).astype(np.float32)
    return output, new_hidden


# revision 14
# speedup vs baseline: 1.5285x; 1.0119x over previous
"""Fused RNN cell on 8 Trainium2 NeuronCores.

Reference computation (fp32):
    combined   = [x, hidden]                      [B=4096, I+H=4096]
    new_hidden = tanh(combined @ W_ih^T + b_ih)   [B, H=2048]
    output     = new_hidden @ W_ho^T + b_ho       [B, O=2048]
    returns (output, new_hidden)

Strategy: data-parallel over the batch — each of the 8 cores processes 512
batch rows with replicated weights; no collectives. All operand layout
transforms (transposes into PE-friendly [K-partition, free] form) happen on
the host so every device DMA is a fat, fully contiguous transfer.

Matmuls run in fp16 (full-rate on the PE — 216 ns per 512-col MM, 1 moving
column/cycle at 2.4 GHz; fp8 DoubleRow was measured at the SAME 512 cycles
per 512-col instruction on this silicon, so an error-compensated fp8
scheme is 1.5x SLOWER than fp16 — don't go back there). PSUM accumulation
is fp32. mm1 produces nh^T [h, b] fp16 tiles in SBUF, which feed mm2
directly as the streaming operand; mm2 produces out^T [o, b] stored fp16.
Outputs are un-transposed and upcast on the host after the gather; b_ho is
added on the host.

The kernel is PE-bound with a gap-free MM stream (768 x 216 ns = 166 us);
total time = ~7.3 us fixed engine-barrier preamble + time-to-first-weight
+ 166 us + drain tail. v2 attacks time-to-first-weight: every per-kp
weight tile is split in half across BOTH HWDGE rings (sync gets h-chunks
0-3, ACT gets 4-7) so the first real matmul can start ~2.5 us after the
rings open instead of ~5.9; the c stream rides GpSimd SWDGE (except the
first 4 ko-chunks, which lead the two HWDGE rings) so the rings carry
nothing but weights during mm1 group 0. nhT stores also ride SWDGE.
outT evictions alternate DVE/ACT, their stores alternate sync/ACT rings,
deferred one group so a store waiting on compute never head-of-line
blocks a load ring. Dummy matmuls at t=0 warm the PE clock gate (HAM) and
preload the ACT tanh table during the initial DMA ramp.
"""

import numpy as np
import ml_dtypes

import concourse.bass as bass
import concourse.mybir as mybir
import concourse.tile as tile
from concourse import bacc, bass_utils

NCORES = 8
B, I, H, O = 4096, 2048, 2048, 2048
BC = B // NCORES          # 512 batch rows per core
K1 = I + H                # mm1 contraction dim, 4096
KO1 = K1 // 128           # 32 k-chunks for mm1
KP8 = 3                   # mm1 k-pair steps (256 k each) run in fp8 DoubleRow
KO16 = KO1 - 2 * KP8      # leading k-chunks run in fp16 (28)
HC = H // 128             # 16 h-chunks
OC = O // 128             # 16 o-chunks
G = 8                     # h/o-chunks per PSUM group (8 banks)
P = 128
SW = 64.0                 # mm1 weight pre-scale (shared by fp16 and fp8 parts)
NWARM = 30                # dummy PE warm-up matmuls
F32 = mybir.dt.float32
F16 = mybir.dt.float16
F8 = mybir.dt.float8e4
AF = mybir.ActivationFunctionType
DR = mybir.MatmulPerfMode.DoubleRow
E4 = ml_dtypes.float8_e4m3fn
NPF16 = np.float16


def _build():
    nc = bacc.Bacc("TRN2", target_bir_lowering=False)

    c = nc.dram_tensor("c", [P, KO16, BC], F16, kind="ExternalInput")
    c8 = nc.dram_tensor("c8", [P, 2 * KP8, BC], F8, kind="ExternalInput")
    w1 = nc.dram_tensor("w1", [P, KO16, HC, P], F16, kind="ExternalInput")
    w18 = nc.dram_tensor("w18", [P, KP8, 2, HC, P], F8, kind="ExternalInput")
    b1 = nc.dram_tensor("b1", [P, HC], F32, kind="ExternalInput")
    w2 = nc.dram_tensor("w2", [P, HC, OC, P], F16, kind="ExternalInput")
    nhT = nc.dram_tensor("nhT", [H, BC], F16, kind="ExternalOutput")
    outT = nc.dram_tensor("outT", [O, BC], F16, kind="ExternalOutput")

    with tile.TileContext(nc) as tc:
        with tc.tile_pool(name="cpool", bufs=1) as cpool, \
             tc.tile_pool(name="wpool", bufs=10) as wpool, \
             tc.tile_pool(name="nhpool", bufs=1) as nhpool, \
             tc.tile_pool(name="opool", bufs=8) as opool, \
             tc.tile_pool(name="bpool", bufs=1) as bpool, \
             tc.tile_pool(name="ps", bufs=8, space="PSUM") as ps:

            # PE warm-up: the HAM clock gate holds the PE at 1.2 GHz until
            # it has been busy ~3.4 µs. Dummy matmuls (no data deps beyond
            # one memset) keep the PE active while the first input tiles
            # stream in, so real matmuls start near 2.4 GHz. The memset
            # rides GpSimd, whose queue opens right after the preamble.
            warm_sb = bpool.tile([P, P], F16)
            nc.gpsimd.memset(warm_sb[:], 0.0)

            b1_sb = bpool.tile([P, HC], F32)
            # b_ih isn't needed until the first group drains (~65 µs);
            # keep it off the HWDGE rings entirely (SWDGE via GpSimd).
            nc.gpsimd.dma_start(b1_sb[:], b1[:])

            c_sb = cpool.tile([P, KO16, BC], F16)
            c8_sb = cpool.tile([P, 2 * KP8, BC], F8)
            nh_sb = nhpool.tile([P, HC, BC], F16)

            # Stores are deferred one group: group g's stores are emitted
            # after group g+1's loads, so when the sync sequencer reaches
            # them the producing compute finished long ago and the ring
            # never head-of-line blocks on a store waiting for compute.
            deferred = []

            def flush_deferred():
                for fn in deferred:
                    fn()
                deferred.clear()

            # mm1: nh^T[h, b] = tanh(W_ih @ combined^T + b_ih)
            # G-sized PSUM groups ping-pong across the 8 banks: while one
            # group's banks drain through ACT, the next group accumulates
            # — group boundaries cost the PE almost nothing.
            for g in range(HC // G):
                psums = [ps.tile([P, BC], F32, tag="ps", name=f"ps{i}")
                         for i in range(G)]
                if g == 0:
                    # The first c and w1 half-tiles land ~2.5 µs after the
                    # rings open; the warm matmuls bridge until then and
                    # start the ~3.4 µs HAM ramp.
                    for _ in range(NWARM):
                        nc.tensor.matmul(
                            psums[G - 1][:, :P], lhsT=warm_sb[:],
                            rhs=warm_sb[:],
                            start=True, stop=True, skip_group_check=True,
                        )
                h0 = g * G
                for ko0 in range(0, KO16, 2):
                    if g == 0:
                        # c rides the ACT HWDGE ring: descriptor pushes for
                        # the first c and w1 tiles then run in parallel on
                        # two queues, and during all of group 0 the sync
                        # ring carries only weights.
                        nc.scalar.dma_start(c_sb[:, ko0:ko0 + 2], c[:, ko0:ko0 + 2])
                        if ko0 == 2:
                            # Preload the ACT tanh table set (~1.3 µs)
                            # during the ramp, not at the first drain.
                            act_warm = bpool.tile([1, 1], F32)
                            nc.scalar.activation(
                                act_warm[:], warm_sb[:1, :1], AF.Tanh)
                        if ko0 == 4:
                            # fp8 tail of c: tiny (512 KiB/8), needed only
                            # at the end of the group — push during ramp.
                            nc.scalar.dma_start(c8_sb[:], c8[:])
                    w1_sb = wpool.tile([P, 2, G, P], F16, tag="w")
                    if g == 0 and ko0 <= 2:
                        # The first two weight tiles are split into two
                        # half pushes so the first matmuls can start after
                        # ~256 KiB instead of ~512 KiB of ring traffic.
                        nc.sync.dma_start(
                            w1_sb[:, :, :4], w1[:, ko0:ko0 + 2, h0:h0 + 4])
                        nc.sync.dma_start(
                            w1_sb[:, :, 4:], w1[:, ko0:ko0 + 2, h0 + 4:h0 + 8])
                    else:
                        nc.sync.dma_start(
                            w1_sb[:], w1[:, ko0:ko0 + 2, h0:h0 + G])
                    for kk in range(2):
                        for i in range(G):
                            nc.tensor.matmul(
                                psums[i][:],
                                lhsT=w1_sb[:, kk, i],
                                rhs=c_sb[:, ko0 + kk],
                                start=(ko0 + kk == 0),
                                stop=False,
                            )
                # fp8 DoubleRow tail: each instruction contracts 256 k
                # (2 paired k-tiles) at the same 512-cycle cost as one
                # fp16 matmul — 2x FLOPs/instruction. Both the fp16 and
                # fp8 partials carry the same x64 weight pre-scale, so
                # they accumulate into the SAME PSUM bank; ACT's
                # scale=1/64 undoes it at eviction.
                for kp in range(KP8):
                    w18_sb = wpool.tile([P, 2, G, P], F8, tag="w", name="w18")
                    nc.sync.dma_start(w18_sb[:], w18[:, kp, :, h0:h0 + G])
                    for i in range(G):
                        nc.tensor.matmul(
                            psums[i][:],
                            lhsT=w18_sb[:, :, i],
                            rhs=c8_sb[:, 2 * kp:2 * kp + 2],
                            start=False,
                            stop=(kp == KP8 - 1),
                            perf_mode=DR,
                        )
                flush_deferred()
                for i in range(G):
                    hc = g * G + i
                    nc.scalar.activation(
                        nh_sb[:, hc], psums[i][:], AF.Tanh,
                        bias=b1_sb[:, hc:hc + 1], scale=1.0 / SW,
                    )
                    # nhT stores ride SWDGE: no HWDGE ring time spent.
                    nc.gpsimd.dma_start(
                        nhT[hc * P:(hc + 1) * P, :], nh_sb[:, hc])

            # mm2: out^T[o, b] = W_ho @ nh^T + b_ho
            # Groups of [8, 4, 2, 2] o-chunks: trailing groups ping-pong
            # through the 8 PSUM banks (no boundary stall) and shrink so
            # the post-last-matmul drain chain is as short as possible.
            for g0, gsz in ((0, 8), (8, 4), (12, 2), (14, 2)):
                psums = [ps.tile([P, BC], F32, tag="ps", name=f"ps{i}")
                         for i in range(gsz)]
                for ho0 in range(0, HC, 2):
                    w2_sb = wpool.tile(
                        [P, 2, G, P], F16, tag="w", name="w2_sb")[:, :, :gsz]
                    nc.sync.dma_start(
                        w2_sb[:], w2[:, ho0:ho0 + 2, g0:g0 + gsz])
                    for kk in range(2):
                        for i in range(gsz):
                            nc.tensor.matmul(
                                psums[i][:],
                                lhsT=w2_sb[:, kk, i],
                                rhs=nh_sb[:, ho0 + kk],
                                start=(ho0 + kk == 0),
                                stop=(ho0 + kk == HC - 1),
                            )
                flush_deferred()
                # Evict PSUM through both DVE and ACT in parallel (raw
                # copies; b_ho is added on the host). ACT-evicted tiles
                # store via the ACT HWDGE ring right behind their copy;
                # DVE-evicted tiles store via the sync ring, deferred one
                # group so the ring never waits on the copy.
                last = (g0 + gsz == OC)
                for i in range(gsz):
                    oc = g0 + i
                    o_sb = opool.tile([P, BC], F16, tag="osb")
                    if i % 2:
                        nc.scalar.activation(o_sb[:], psums[i][:], AF.Copy)
                        nc.scalar.dma_start(
                            outT[oc * P:(oc + 1) * P, :], o_sb[:])
                    else:
                        nc.vector.tensor_copy(o_sb[:], psums[i][:])
                        st = (lambda oc=oc, o_sb=o_sb: nc.sync.dma_start(
                            outT[oc * P:(oc + 1) * P, :], o_sb[:]))
                        if last:
                            st()      # no deferral on the final group
                        else:
                            deferred.append(st)
            flush_deferred()

    nc.compile()
    return nc


def _shard_inputs(x, hidden, W_ih, b_ih, W_ho, b_ho):
    combined = np.concatenate([x, hidden], axis=1)  # [B, K1]
    K16 = KO16 * P                                  # fp16 k-range (3584)
    W1s = W_ih.astype(np.float32) * SW
    w1L = np.ascontiguousarray(
        W1s[:, :K16].reshape(HC, P, KO16, P).transpose(3, 2, 0, 1)
    ).astype(NPF16)  # [ki, ko, hc, h]
    w18L = np.ascontiguousarray(
        np.clip(W1s[:, K16:], -240, 240).astype(E4)
        .reshape(HC, P, KP8, 2, P).transpose(4, 2, 3, 0, 1)
    )  # [ki, kp, kk, hc, h]
    w2L = np.ascontiguousarray(
        W_ho.reshape(OC, P, HC, P).transpose(3, 2, 0, 1)
    ).astype(NPF16)  # [hi, ho, oc, o]
    b1L = np.ascontiguousarray(b_ih.reshape(HC, P).T)
    in_maps = []
    for cix in range(NCORES):
        cc = combined[cix * BC:(cix + 1) * BC]  # [BC, K1]
        cL = np.ascontiguousarray(
            cc[:, :K16].reshape(BC, KO16, P).transpose(2, 1, 0)).astype(NPF16)
        c8L = np.ascontiguousarray(
            np.clip(cc[:, K16:], -240, 240).astype(E4)
            .reshape(BC, 2 * KP8, P).transpose(2, 1, 0))
        in_maps.append(
            {"c": cL, "c8": c8L, "w1": w1L, "w18": w18L,
             "b1": b1L, "w2": w2L}
        )
    return in_maps


def _run(in_maps, **kwargs):
    nc = _build()
    return bass_utils.run_bass_kernel_spmd(
        nc, in_maps, core_ids=list(range(NCORES)), **kwargs
    )


def kernel(x, hidden, W_ih, b_ih, W_ho, b_ho):
    x = np.asarray(x, dtype=np.float32)
    hidden = np.asarray(hidden, dtype=np.float32)
    W_ih = np.asarray(W_ih, dtype=np.float32)
    b_ih = np.asarray(b_ih, dtype=np.float32)
    W_ho = np.asarray(W_ho, dtype=np.float32)
    b_ho = np.asarray(b_ho, dtype=np.float32)

    in_maps = _shard_inputs(x, hidden, W_ih, b_ih, W_ho, b_ho)
    res = _run(in_maps)
    output = np.concatenate(
        [r["outT"].T.astype(np.float32) for r in res.results], axis=0) + b_ho
    new_hidden = np.concatenate(
        [r["nhT"].T for r in res.results], axis=0).astype(np.float32)
    return output, new_hidden


# revision 16
# speedup vs baseline: 1.5313x; 1.0018x over previous
"""Fused RNN cell on 8 Trainium2 NeuronCores.

Reference computation (fp32):
    combined   = [x, hidden]                      [B=4096, I+H=4096]
    new_hidden = tanh(combined @ W_ih^T + b_ih)   [B, H=2048]
    output     = new_hidden @ W_ho^T + b_ho       [B, O=2048]
    returns (output, new_hidden)

Strategy: data-parallel over the batch — each of the 8 cores processes 512
batch rows with replicated weights; no collectives. All operand layout
transforms (transposes into PE-friendly [K-partition, free] form) happen on
the host so every device DMA is a fat, fully contiguous transfer.

Matmuls run in fp16 (full-rate on the PE — 216 ns per 512-col MM, 1 moving
column/cycle at 2.4 GHz; fp8 DoubleRow was measured at the SAME 512 cycles
per 512-col instruction on this silicon, so an error-compensated fp8
scheme is 1.5x SLOWER than fp16 — don't go back there). PSUM accumulation
is fp32. mm1 produces nh^T [h, b] fp16 tiles in SBUF, which feed mm2
directly as the streaming operand; mm2 produces out^T [o, b] stored fp16.
Outputs are un-transposed and upcast on the host after the gather; b_ho is
added on the host.

The kernel is PE-bound with a gap-free MM stream (768 x 216 ns = 166 us);
total time = ~7.3 us fixed engine-barrier preamble + time-to-first-weight
+ 166 us + drain tail. v2 attacks time-to-first-weight: every per-kp
weight tile is split in half across BOTH HWDGE rings (sync gets h-chunks
0-3, ACT gets 4-7) so the first real matmul can start ~2.5 us after the
rings open instead of ~5.9; the c stream rides GpSimd SWDGE (except the
first 4 ko-chunks, which lead the two HWDGE rings) so the rings carry
nothing but weights during mm1 group 0. nhT stores also ride SWDGE.
outT evictions alternate DVE/ACT, their stores alternate sync/ACT rings,
deferred one group so a store waiting on compute never head-of-line
blocks a load ring. Dummy matmuls at t=0 warm the PE clock gate (HAM) and
preload the ACT tanh table during the initial DMA ramp.
"""

import numpy as np
import ml_dtypes

import concourse.bass as bass
import concourse.mybir as mybir
import concourse.tile as tile
from concourse import bacc, bass_utils

NCORES = 8
B, I, H, O = 4096, 2048, 2048, 2048
BC = B // NCORES          # 512 batch rows per core
K1 = I + H                # mm1 contraction dim, 4096
KO1 = K1 // 128           # 32 k-chunks for mm1
KP8 = 3                   # mm1 k-pair steps (256 k each) run in fp8 DoubleRow
KO16 = KO1 - 2 * KP8      # leading k-chunks run in fp16 (28)
HC = H // 128             # 16 h-chunks
OC = O // 128             # 16 o-chunks
G = 8                     # h/o-chunks per PSUM group (8 banks)
P = 128
SW = 64.0                 # mm1 weight pre-scale (shared by fp16 and fp8 parts)
NWARM = 30                # dummy PE warm-up matmuls
F32 = mybir.dt.float32
F16 = mybir.dt.float16
F8 = mybir.dt.float8e4
AF = mybir.ActivationFunctionType
DR = mybir.MatmulPerfMode.DoubleRow
E4 = ml_dtypes.float8_e4m3fn
NPF16 = np.float16


def _build():
    nc = bacc.Bacc("TRN2", target_bir_lowering=False)

    c = nc.dram_tensor("c", [P, KO16, BC], F16, kind="ExternalInput")
    c8 = nc.dram_tensor("c8", [P, 2 * KP8, BC], F8, kind="ExternalInput")
    w1 = nc.dram_tensor("w1", [P, KO16, HC, P], F16, kind="ExternalInput")
    w18 = nc.dram_tensor("w18", [P, KP8, 2, HC, P], F8, kind="ExternalInput")
    b1 = nc.dram_tensor("b1", [P, HC], F32, kind="ExternalInput")
    w2 = nc.dram_tensor("w2", [P, HC, OC, P], F16, kind="ExternalInput")
    nhT = nc.dram_tensor("nhT", [H, BC], F16, kind="ExternalOutput")
    outT = nc.dram_tensor("outT", [O, BC], F16, kind="ExternalOutput")

    with tile.TileContext(nc) as tc:
        with tc.tile_pool(name="cpool", bufs=1) as cpool, \
             tc.tile_pool(name="wpool", bufs=10) as wpool, \
             tc.tile_pool(name="nhpool", bufs=1) as nhpool, \
             tc.tile_pool(name="opool", bufs=8) as opool, \
             tc.tile_pool(name="bpool", bufs=1) as bpool, \
             tc.tile_pool(name="ps", bufs=8, space="PSUM") as ps:

            # PE warm-up: the HAM clock gate holds the PE at 1.2 GHz until
            # it has been busy ~3.4 µs. Dummy matmuls (no data deps beyond
            # one memset) keep the PE active while the first input tiles
            # stream in, so real matmuls start near 2.4 GHz. The memset
            # rides GpSimd, whose queue opens right after the preamble.
            warm_sb = bpool.tile([P, P], F16)
            nc.gpsimd.memset(warm_sb[:], 0.0)

            c_sb = cpool.tile([P, KO16, BC], F16)
            c8_sb = cpool.tile([P, 2 * KP8, BC], F8)
            # The first 8 c chunks ride GpSimd SWDGE (land ~10-12.5 µs,
            # ahead of the weight halves) so both HWDGE rings open with
            # weight tiles and nothing else.
            for ko0 in range(0, 8, 2):
                nc.gpsimd.dma_start(c_sb[:, ko0:ko0 + 2], c[:, ko0:ko0 + 2])

            b1_sb = bpool.tile([P, HC], F32)
            # b_ih isn't needed until the first group drains (~65 µs);
            # keep it off the HWDGE rings entirely (SWDGE via GpSimd).
            nc.gpsimd.dma_start(b1_sb[:], b1[:])

            nh_sb = nhpool.tile([P, HC, BC], F16)

            # Stores are deferred one group: group g's stores are emitted
            # after group g+1's loads, so when the sync sequencer reaches
            # them the producing compute finished long ago and the ring
            # never head-of-line blocks on a store waiting for compute.
            deferred = []

            def flush_deferred():
                for fn in deferred:
                    fn()
                deferred.clear()

            # mm1: nh^T[h, b] = tanh(W_ih @ combined^T + b_ih)
            # G-sized PSUM groups ping-pong across the 8 banks: while one
            # group's banks drain through ACT, the next group accumulates
            # — group boundaries cost the PE almost nothing.
            for g in range(HC // G):
                psums = [ps.tile([P, BC], F32, tag="ps", name=f"ps{i}")
                         for i in range(G)]
                if g == 0:
                    # The first c and w1 half-tiles land ~2.5 µs after the
                    # rings open; the warm matmuls bridge until then and
                    # start the ~3.4 µs HAM ramp.
                    for _ in range(NWARM):
                        nc.tensor.matmul(
                            psums[G - 1][:, :P], lhsT=warm_sb[:],
                            rhs=warm_sb[:],
                            start=True, stop=True, skip_group_check=True,
                        )
                h0 = g * G
                for ko0 in range(0, KO16, 2):
                    if g == 0 and ko0 == 8:
                        # Preload the ACT tanh table set (~1.3 µs) during
                        # the ramp, then queue the remaining c behind it.
                        act_warm = bpool.tile([1, 1], F32)
                        nc.scalar.activation(
                            act_warm[:], warm_sb[:1, :1], AF.Tanh)
                    if g == 0 and ko0 >= 8:
                        # c beyond the SWDGE-carried first 8 chunks rides
                        # the ACT HWDGE ring, well ahead of consumption.
                        nc.scalar.dma_start(c_sb[:, ko0:ko0 + 2], c[:, ko0:ko0 + 2])
                        if ko0 == 10:
                            # fp8 tail of c: tiny, needed only at the end
                            # of the group — push during the ramp.
                            nc.scalar.dma_start(c8_sb[:], c8[:])
                    w1_sb = wpool.tile([P, 2, G, P], F16, tag="w")
                    if g == 0 and ko0 <= 6:
                        # The first four weight tiles are split into two
                        # half pushes running on BOTH rings in parallel,
                        # so each lands in ~half the ring time and the MM
                        # stream starts ~1 µs after the first half.
                        nc.sync.dma_start(
                            w1_sb[:, :, :4], w1[:, ko0:ko0 + 2, h0:h0 + 4])
                        nc.scalar.dma_start(
                            w1_sb[:, :, 4:], w1[:, ko0:ko0 + 2, h0 + 4:h0 + 8])
                    else:
                        nc.sync.dma_start(
                            w1_sb[:], w1[:, ko0:ko0 + 2, h0:h0 + G])
                    for kk in range(2):
                        for i in range(G):
                            nc.tensor.matmul(
                                psums[i][:],
                                lhsT=w1_sb[:, kk, i],
                                rhs=c_sb[:, ko0 + kk],
                                start=(ko0 + kk == 0),
                                stop=False,
                            )
                # fp8 DoubleRow tail: each instruction contracts 256 k
                # (2 paired k-tiles) at the same 512-cycle cost as one
                # fp16 matmul — 2x FLOPs/instruction. Both the fp16 and
                # fp8 partials carry the same x64 weight pre-scale, so
                # they accumulate into the SAME PSUM bank; ACT's
                # scale=1/64 undoes it at eviction.
                for kp in range(KP8):
                    w18_sb = wpool.tile([P, 2, G, P], F8, tag="w", name="w18")
                    nc.sync.dma_start(w18_sb[:], w18[:, kp, :, h0:h0 + G])
                    for i in range(G):
                        nc.tensor.matmul(
                            psums[i][:],
                            lhsT=w18_sb[:, :, i],
                            rhs=c8_sb[:, 2 * kp:2 * kp + 2],
                            start=False,
                            stop=(kp == KP8 - 1),
                            perf_mode=DR,
                        )
                flush_deferred()
                for i in range(G):
                    hc = g * G + i
                    nc.scalar.activation(
                        nh_sb[:, hc], psums[i][:], AF.Tanh,
                        bias=b1_sb[:, hc:hc + 1], scale=1.0 / SW,
                    )
                    # nhT stores ride SWDGE: no HWDGE ring time spent.
                    nc.gpsimd.dma_start(
                        nhT[hc * P:(hc + 1) * P, :], nh_sb[:, hc])

            # mm2: out^T[o, b] = W_ho @ nh^T + b_ho
            # Groups of [8, 4, 2, 2] o-chunks: trailing groups ping-pong
            # through the 8 PSUM banks (no boundary stall) and shrink so
            # the post-last-matmul drain chain is as short as possible.
            for g0, gsz in ((0, 8), (8, 4), (12, 2), (14, 2)):
                psums = [ps.tile([P, BC], F32, tag="ps", name=f"ps{i}")
                         for i in range(gsz)]
                for ho0 in range(0, HC, 2):
                    w2_sb = wpool.tile(
                        [P, 2, G, P], F16, tag="w", name="w2_sb")[:, :, :gsz]
                    nc.sync.dma_start(
                        w2_sb[:], w2[:, ho0:ho0 + 2, g0:g0 + gsz])
                    for kk in range(2):
                        for i in range(gsz):
                            nc.tensor.matmul(
                                psums[i][:],
                                lhsT=w2_sb[:, kk, i],
                                rhs=nh_sb[:, ho0 + kk],
                                start=(ho0 + kk == 0),
                                stop=(ho0 + kk == HC - 1),
                            )
                flush_deferred()
                # Evict PSUM through both DVE and ACT in parallel (raw
                # copies; b_ho is added on the host). ACT-evicted tiles
                # store via the ACT HWDGE ring right behind their copy;
                # DVE-evicted tiles store via the sync ring, deferred one
                # group so the ring never waits on the copy.
                last = (g0 + gsz == OC)
                for i in range(gsz):
                    oc = g0 + i
                    o_sb = opool.tile([P, BC], F16, tag="osb")
                    if i % 2:
                        nc.scalar.activation(o_sb[:], psums[i][:], AF.Copy)
                        nc.scalar.dma_start(
                            outT[oc * P:(oc + 1) * P, :], o_sb[:])
                    else:
                        nc.vector.tensor_copy(o_sb[:], psums[i][:])
                        st = (lambda oc=oc, o_sb=o_sb: nc.sync.dma_start(
                            outT[oc * P:(oc + 1) * P, :], o_sb[:]))
                        if last:
                            st()      # no deferral on the final group
                        else:
                            deferred.append(st)
            flush_deferred()

    nc.compile()
    return nc


def _shard_inputs(x, hidden, W_ih, b_ih, W_ho, b_ho):
    combined = np.concatenate([x, hidden], axis=1)  # [B, K1]
    K16 = KO16 * P                                  # fp16 k-range (3584)
    W1s = W_ih.astype(np.float32) * SW
    w1L = np.ascontiguousarray(
        W1s[:, :K16].reshape(HC, P, KO16, P).transpose(3, 2, 0, 1)
    ).astype(NPF16)  # [ki, ko, hc, h]
    w18L = np.ascontiguousarray(
        np.clip(W1s[:, K16:], -240, 240).astype(E4)
        .reshape(HC, P, KP8, 2, P).transpose(4, 2, 3, 0, 1)
    )  # [ki, kp, kk, hc, h]
    w2L = np.ascontiguousarray(
        W_ho.reshape(OC, P, HC, P).transpose(3, 2, 0, 1)
    ).astype(NPF16)  # [hi, ho, oc, o]
    b1L = np.ascontiguousarray(b_ih.reshape(HC, P).T)
    in_maps = []
    for cix in range(NCORES):
        cc = combined[cix * BC:(cix + 1) * BC]  # [BC, K1]
        cL = np.ascontiguousarray(
            cc[:, :K16].reshape(BC, KO16, P).transpose(2, 1, 0)).astype(NPF16)
        c8L = np.ascontiguousarray(
            np.clip(cc[:, K16:], -240, 240).astype(E4)
            .reshape(BC, 2 * KP8, P).transpose(2, 1, 0))
        in_maps.append(
            {"c": cL, "c8": c8L, "w1": w1L, "w18": w18L,
             "b1": b1L, "w2": w2L}
        )
    return in_maps


def _run(in_maps, **kwargs):
    nc = _build()
    return bass_utils.run_bass_kernel_spmd(
        nc, in_maps, core_ids=list(range(NCORES)), **kwargs
    )


def kernel(x, hidden, W_ih, b_ih, W_ho, b_ho):
    x = np.asarray(x, dtype=np.float32)
    hidden = np.asarray(hidden, dtype=np.float32)
    W_ih = np.asarray(W_ih, dtype=np.float32)
    b_ih = np.asarray(b_ih, dtype=np.float32)
    W_ho = np.asarray(W_ho, dtype=np.float32)
    b_ho = np.asarray(b_ho, dtype=np.float32)

    in_maps = _shard_inputs(x, hidden, W_ih, b_ih, W_ho, b_ho)
    res = _run(in_maps)
    output = np.concatenate(
        [r["outT"].T.astype(np.float32) for r in res.results], axis=0) + b_ho
    new_hidden = np.concatenate(
        [r["nhT"].T for r in res.results], axis=0).astype(np.float32)
    return output, new_hidden


# revision 18
# speedup vs baseline: 1.5325x; 1.0008x over previous
"""Fused RNN cell on 8 Trainium2 NeuronCores.

Reference computation (fp32):
    combined   = [x, hidden]                      [B=4096, I+H=4096]
    new_hidden = tanh(combined @ W_ih^T + b_ih)   [B, H=2048]
    output     = new_hidden @ W_ho^T + b_ho       [B, O=2048]
    returns (output, new_hidden)

Strategy: data-parallel over the batch — each of the 8 cores processes 512
batch rows with replicated weights; no collectives. All operand layout
transforms (transposes into PE-friendly [K-partition, free] form) happen on
the host so every device DMA is a fat, fully contiguous transfer.

Matmuls run in fp16 (full-rate on the PE — 216 ns per 512-col MM, 1 moving
column/cycle at 2.4 GHz; fp8 DoubleRow was measured at the SAME 512 cycles
per 512-col instruction on this silicon, so an error-compensated fp8
scheme is 1.5x SLOWER than fp16 — don't go back there). PSUM accumulation
is fp32. mm1 produces nh^T [h, b] fp16 tiles in SBUF, which feed mm2
directly as the streaming operand; mm2 produces out^T [o, b] stored fp16.
Outputs are un-transposed and upcast on the host after the gather; b_ho is
added on the host.

The kernel is PE-bound with a gap-free MM stream (768 x 216 ns = 166 us);
total time = ~7.3 us fixed engine-barrier preamble + time-to-first-weight
+ 166 us + drain tail. v2 attacks time-to-first-weight: every per-kp
weight tile is split in half across BOTH HWDGE rings (sync gets h-chunks
0-3, ACT gets 4-7) so the first real matmul can start ~2.5 us after the
rings open instead of ~5.9; the c stream rides GpSimd SWDGE (except the
first 4 ko-chunks, which lead the two HWDGE rings) so the rings carry
nothing but weights during mm1 group 0. nhT stores also ride SWDGE.
outT evictions alternate DVE/ACT, their stores alternate sync/ACT rings,
deferred one group so a store waiting on compute never head-of-line
blocks a load ring. Dummy matmuls at t=0 warm the PE clock gate (HAM) and
preload the ACT tanh table during the initial DMA ramp.
"""

import numpy as np
import ml_dtypes

import concourse.bass as bass
import concourse.mybir as mybir
import concourse.tile as tile
from concourse import bacc, bass_utils

NCORES = 8
B, I, H, O = 4096, 2048, 2048, 2048
BC = B // NCORES          # 512 batch rows per core
K1 = I + H                # mm1 contraction dim, 4096
KO1 = K1 // 128           # 32 k-chunks for mm1
KP8 = 3                   # mm1 k-pair steps (256 k each) run in fp8 DoubleRow
KO16 = KO1 - 2 * KP8      # leading k-chunks run in fp16 (28)
HC = H // 128             # 16 h-chunks
OC = O // 128             # 16 o-chunks
G = 8                     # h/o-chunks per PSUM group (8 banks)
P = 128
SW = 64.0                 # mm1 weight pre-scale (shared by fp16 and fp8 parts)
NWARM = 30                # dummy PE warm-up matmuls
F32 = mybir.dt.float32
F16 = mybir.dt.float16
F8 = mybir.dt.float8e4
AF = mybir.ActivationFunctionType
DR = mybir.MatmulPerfMode.DoubleRow
E4 = ml_dtypes.float8_e4m3fn
NPF16 = np.float16


def _build():
    nc = bacc.Bacc("TRN2", target_bir_lowering=False)

    c = nc.dram_tensor("c", [P, KO16, BC], F16, kind="ExternalInput")
    c8 = nc.dram_tensor("c8", [P, 2 * KP8, BC], F8, kind="ExternalInput")
    w1 = nc.dram_tensor("w1", [P, KO16, HC, P], F16, kind="ExternalInput")
    w18 = nc.dram_tensor("w18", [P, KP8, 2, HC, P], F8, kind="ExternalInput")
    b1 = nc.dram_tensor("b1", [P, HC], F32, kind="ExternalInput")
    w2 = nc.dram_tensor("w2", [P, HC, OC, P], F16, kind="ExternalInput")
    nhT = nc.dram_tensor("nhT", [H, BC], F16, kind="ExternalOutput")
    outT = nc.dram_tensor("outT", [O, BC], F16, kind="ExternalOutput")

    with tile.TileContext(nc) as tc:
        with tc.tile_pool(name="cpool", bufs=1) as cpool, \
             tc.tile_pool(name="wpool", bufs=10) as wpool, \
             tc.tile_pool(name="nhpool", bufs=1) as nhpool, \
             tc.tile_pool(name="opool", bufs=8) as opool, \
             tc.tile_pool(name="bpool", bufs=1) as bpool, \
             tc.tile_pool(name="ps", bufs=8, space="PSUM") as ps:

            # PE warm-up: the HAM clock gate holds the PE at 1.2 GHz until
            # it has been busy ~3.4 µs. Dummy matmuls (no data deps beyond
            # one memset) keep the PE active while the first input tiles
            # stream in, so real matmuls start near 2.4 GHz. The memset
            # rides GpSimd, whose queue opens right after the preamble.
            warm_sb = bpool.tile([P, P], F16)
            nc.gpsimd.memset(warm_sb[:], 0.0)

            b1_sb = bpool.tile([P, HC], F32)
            # b_ih isn't needed until the first group drains (~65 µs);
            # keep it off the HWDGE rings entirely (SWDGE via GpSimd).
            nc.gpsimd.dma_start(b1_sb[:], b1[:])

            c_sb = cpool.tile([P, KO16, BC], F16)
            c8_sb = cpool.tile([P, 2 * KP8, BC], F8)
            nh_sb = nhpool.tile([P, HC, BC], F16)

            # Stores are deferred one group: group g's stores are emitted
            # after group g+1's loads, so when the sync sequencer reaches
            # them the producing compute finished long ago and the ring
            # never head-of-line blocks on a store waiting for compute.
            deferred = []

            def flush_deferred():
                for fn in deferred:
                    fn()
                deferred.clear()

            # mm1: nh^T[h, b] = tanh(W_ih @ combined^T + b_ih)
            # G-sized PSUM groups ping-pong across the 8 banks: while one
            # group's banks drain through ACT, the next group accumulates
            # — group boundaries cost the PE almost nothing.
            for g in range(HC // G):
                psums = [ps.tile([P, BC], F32, tag="ps", name=f"ps{i}")
                         for i in range(G)]
                if g == 0:
                    # The first c and w1 half-tiles land ~2.5 µs after the
                    # rings open; the warm matmuls bridge until then and
                    # start the ~3.4 µs HAM ramp.
                    for _ in range(NWARM):
                        nc.tensor.matmul(
                            psums[G - 1][:, :P], lhsT=warm_sb[:],
                            rhs=warm_sb[:],
                            start=True, stop=True, skip_group_check=True,
                        )
                h0 = g * G
                for ko0 in range(0, KO16, 2):
                    if g == 0:
                        # c rides the ACT HWDGE ring: descriptor pushes for
                        # the first c and w1 tiles then run in parallel on
                        # two queues, and during all of group 0 the sync
                        # ring carries only weights.
                        nc.scalar.dma_start(c_sb[:, ko0:ko0 + 2], c[:, ko0:ko0 + 2])
                        if ko0 == 2:
                            # Preload the ACT tanh table set (~1.3 µs)
                            # during the ramp, not at the first drain.
                            act_warm = bpool.tile([1, 1], F32)
                            nc.scalar.activation(
                                act_warm[:], warm_sb[:1, :1], AF.Tanh)
                        if ko0 == 4:
                            # fp8 tail of c: tiny, needed only at the end
                            # of the group — push during the ramp.
                            nc.scalar.dma_start(c8_sb[:], c8[:])
                    w1_sb = wpool.tile([P, 2, G, P], F16, tag="w")
                    if g == 0 and ko0 <= 2:
                        # The first two weight tiles are split into two
                        # half pushes so the first matmuls can start after
                        # ~256 KiB instead of ~512 KiB of ring traffic.
                        nc.sync.dma_start(
                            w1_sb[:, :, :4], w1[:, ko0:ko0 + 2, h0:h0 + 4])
                        nc.sync.dma_start(
                            w1_sb[:, :, 4:], w1[:, ko0:ko0 + 2, h0 + 4:h0 + 8])
                    else:
                        nc.sync.dma_start(
                            w1_sb[:], w1[:, ko0:ko0 + 2, h0:h0 + G])
                    for kk in range(2):
                        for i in range(G):
                            nc.tensor.matmul(
                                psums[i][:],
                                lhsT=w1_sb[:, kk, i],
                                rhs=c_sb[:, ko0 + kk],
                                start=(ko0 + kk == 0),
                                stop=False,
                            )
                # fp8 DoubleRow tail: each instruction contracts 256 k
                # (2 paired k-tiles) at the same 512-cycle cost as one
                # fp16 matmul — 2x FLOPs/instruction. Both the fp16 and
                # fp8 partials carry the same x64 weight pre-scale, so
                # they accumulate into the SAME PSUM bank; ACT's
                # scale=1/64 undoes it at eviction.
                for kp in range(KP8):
                    w18_sb = wpool.tile([P, 2, G, P], F8, tag="w", name="w18")
                    nc.sync.dma_start(w18_sb[:], w18[:, kp, :, h0:h0 + G])
                    for i in range(G):
                        nc.tensor.matmul(
                            psums[i][:],
                            lhsT=w18_sb[:, :, i],
                            rhs=c8_sb[:, 2 * kp:2 * kp + 2],
                            start=False,
                            stop=(kp == KP8 - 1),
                            perf_mode=DR,
                        )
                flush_deferred()
                for i in range(G):
                    hc = g * G + i
                    nc.scalar.activation(
                        nh_sb[:, hc], psums[i][:], AF.Tanh,
                        bias=b1_sb[:, hc:hc + 1], scale=1.0 / SW,
                    )
                    # nhT stores ride SWDGE: no HWDGE ring time spent.
                    nc.gpsimd.dma_start(
                        nhT[hc * P:(hc + 1) * P, :], nh_sb[:, hc])

            # mm2: out^T[o, b] = W_ho @ nh^T + b_ho
            # Groups of [8, 4, 2, 2] o-chunks: trailing groups ping-pong
            # through the 8 PSUM banks (no boundary stall) and shrink so
            # the post-last-matmul drain chain is as short as possible.
            for g0, gsz in ((0, 8), (8, 4), (12, 2), (14, 2)):
                psums = [ps.tile([P, BC], F32, tag="ps", name=f"ps{i}")
                         for i in range(gsz)]
                for ho0 in range(0, HC, 2):
                    w2_sb = wpool.tile(
                        [P, 2, G, P], F16, tag="w", name="w2_sb")[:, :, :gsz]
                    nc.sync.dma_start(
                        w2_sb[:], w2[:, ho0:ho0 + 2, g0:g0 + gsz])
                    for kk in range(2):
                        for i in range(gsz):
                            nc.tensor.matmul(
                                psums[i][:],
                                lhsT=w2_sb[:, kk, i],
                                rhs=nh_sb[:, ho0 + kk],
                                start=(ho0 + kk == 0),
                                stop=(ho0 + kk == HC - 1),
                            )
                flush_deferred()
                # Evict PSUM through both DVE and ACT in parallel (raw
                # copies; b_ho is added on the host). ACT-evicted tiles
                # store via the ACT HWDGE ring right behind their copy;
                # DVE-evicted tiles store via the sync ring, deferred one
                # group so the ring never waits on the copy.
                last = (g0 + gsz == OC)
                for i in range(gsz):
                    oc = g0 + i
                    o_sb = opool.tile([P, BC], F16, tag="osb")
                    if i % 2:
                        nc.scalar.activation(o_sb[:], psums[i][:], AF.Copy)
                        nc.scalar.dma_start(
                            outT[oc * P:(oc + 1) * P, :], o_sb[:])
                    else:
                        nc.vector.tensor_copy(o_sb[:], psums[i][:])
                        st = (lambda oc=oc, o_sb=o_sb: nc.sync.dma_start(
                            outT[oc * P:(oc + 1) * P, :], o_sb[:]))
                        if last:
                            st()      # no deferral on the final group
                        else:
                            deferred.append(st)
            flush_deferred()

    nc.compile()
    return nc


def _shard_inputs(x, hidden, W_ih, b_ih, W_ho, b_ho):
    combined = np.concatenate([x, hidden], axis=1)  # [B, K1]
    K16 = KO16 * P                                  # fp16 k-range (3584)
    W1s = W_ih.astype(np.float32) * SW
    w1L = np.ascontiguousarray(
        W1s[:, :K16].reshape(HC, P, KO16, P).transpose(3, 2, 0, 1)
    ).astype(NPF16)  # [ki, ko, hc, h]
    w18L = np.ascontiguousarray(
        np.clip(W1s[:, K16:], -240, 240).astype(E4)
        .reshape(HC, P, KP8, 2, P).transpose(4, 2, 3, 0, 1)
    )  # [ki, kp, kk, hc, h]
    w2L = np.ascontiguousarray(
        W_ho.reshape(OC, P, HC, P).transpose(3, 2, 0, 1)
    ).astype(NPF16)  # [hi, ho, oc, o]
    b1L = np.ascontiguousarray(b_ih.reshape(HC, P).T)
    in_maps = []
    for cix in range(NCORES):
        cc = combined[cix * BC:(cix + 1) * BC]  # [BC, K1]
        cL = np.ascontiguousarray(
            cc[:, :K16].reshape(BC, KO16, P).transpose(2, 1, 0)).astype(NPF16)
        c8L = np.ascontiguousarray(
            np.clip(cc[:, K16:], -240, 240).astype(E4)
            .reshape(BC, 2 * KP8, P).transpose(2, 1, 0))
        in_maps.append(
            {"c": cL, "c8": c8L, "w1": w1L, "w18": w18L,
             "b1": b1L, "w2": w2L}
        )
    return in_maps


def _run(in_maps, **kwargs):
    nc = _build()
    return bass_utils.run_bass_kernel_spmd(
        nc, in_maps, core_ids=list(range(NCORES)), **kwargs
    )


def kernel(x, hidden, W_ih, b_ih, W_ho, b_ho):
    x = np.asarray(x, dtype=np.float32)
    hidden = np.asarray(hidden, dtype=np.float32)
    W_ih = np.asarray(W_ih, dtype=np.float32)
    b_ih = np.asarray(b_ih, dtype=np.float32)
    W_ho = np.asarray(W_ho, dtype=np.float32)
    b_ho = np.asarray(b_ho, dtype=np.float32)

    in_maps = _shard_inputs(x, hidden, W_ih, b_ih, W_ho, b_ho)
    res = _run(in_maps)
    output = np.concatenate(
        [r["outT"].T.astype(np.float32) for r in res.results], axis=0) + b_ho
    new_hidden = np.concatenate(
        [r["nhT"].T for r in res.results], axis=0).astype(np.float32)
    return output, new_hidden


# revision 19
# speedup vs baseline: 1.5431x; 1.0069x over previous
"""Fused RNN cell on 8 Trainium2 NeuronCores.

Reference computation (fp32):
    combined   = [x, hidden]                      [B=4096, I+H=4096]
    new_hidden = tanh(combined @ W_ih^T + b_ih)   [B, H=2048]
    output     = new_hidden @ W_ho^T + b_ho       [B, O=2048]
    returns (output, new_hidden)

Strategy: data-parallel over the batch — each of the 8 cores processes 512
batch rows with replicated weights; no collectives. All operand layout
transforms (transposes into PE-friendly [K-partition, free] form) happen on
the host so every device DMA is a fat, fully contiguous transfer.

Matmuls run in fp16 (full-rate on the PE — 216 ns per 512-col MM, 1 moving
column/cycle at 2.4 GHz; fp8 DoubleRow was measured at the SAME 512 cycles
per 512-col instruction on this silicon, so an error-compensated fp8
scheme is 1.5x SLOWER than fp16 — don't go back there). PSUM accumulation
is fp32. mm1 produces nh^T [h, b] fp16 tiles in SBUF, which feed mm2
directly as the streaming operand; mm2 produces out^T [o, b] stored fp16.
Outputs are un-transposed and upcast on the host after the gather; b_ho is
added on the host.

The kernel is PE-bound with a gap-free MM stream (768 x 216 ns = 166 us);
total time = ~7.3 us fixed engine-barrier preamble + time-to-first-weight
+ 166 us + drain tail. v2 attacks time-to-first-weight: every per-kp
weight tile is split in half across BOTH HWDGE rings (sync gets h-chunks
0-3, ACT gets 4-7) so the first real matmul can start ~2.5 us after the
rings open instead of ~5.9; the c stream rides GpSimd SWDGE (except the
first 4 ko-chunks, which lead the two HWDGE rings) so the rings carry
nothing but weights during mm1 group 0. nhT stores also ride SWDGE.
outT evictions alternate DVE/ACT, their stores alternate sync/ACT rings,
deferred one group so a store waiting on compute never head-of-line
blocks a load ring. Dummy matmuls at t=0 warm the PE clock gate (HAM) and
preload the ACT tanh table during the initial DMA ramp.
"""

import numpy as np
import ml_dtypes

import concourse.bass as bass
import concourse.mybir as mybir
import concourse.tile as tile
from concourse import bacc, bass_utils

NCORES = 8
B, I, H, O = 4096, 2048, 2048, 2048
BC = B // NCORES          # 512 batch rows per core
K1 = I + H                # mm1 contraction dim, 4096
KO1 = K1 // 128           # 32 k-chunks for mm1
KP8 = 3                   # mm1 k-pair steps (256 k each) run in fp8 DoubleRow
KO16 = KO1 - 2 * KP8      # leading k-chunks run in fp16 (28)
HC = H // 128             # 16 h-chunks
OC = O // 128             # 16 o-chunks
G = 8                     # h/o-chunks per PSUM group (8 banks)
P = 128
SW = 64.0                 # mm1 weight pre-scale (shared by fp16 and fp8 parts)
NWARM = 30                # dummy PE warm-up matmuls
F32 = mybir.dt.float32
F16 = mybir.dt.float16
F8 = mybir.dt.float8e4
AF = mybir.ActivationFunctionType
DR = mybir.MatmulPerfMode.DoubleRow
E4 = ml_dtypes.float8_e4m3fn
NPF16 = np.float16


def _build():
    nc = bacc.Bacc("TRN2", target_bir_lowering=False)

    c = nc.dram_tensor("c", [P, KO16, BC], F16, kind="ExternalInput")
    c8 = nc.dram_tensor("c8", [P, 2 * KP8, BC], F8, kind="ExternalInput")
    w1 = nc.dram_tensor("w1", [P, KO16, HC, P], F16, kind="ExternalInput")
    w18 = nc.dram_tensor("w18", [P, KP8, 2, HC, P], F8, kind="ExternalInput")
    b1 = nc.dram_tensor("b1", [P, HC], F32, kind="ExternalInput")
    w2 = nc.dram_tensor("w2", [P, HC, OC, P], F16, kind="ExternalInput")
    nhT = nc.dram_tensor("nhT", [H, BC], F16, kind="ExternalOutput")
    outT = nc.dram_tensor("outT", [O, BC], F16, kind="ExternalOutput")

    with tile.TileContext(nc) as tc:
        with tc.tile_pool(name="cpool", bufs=1) as cpool, \
             tc.tile_pool(name="wpool", bufs=10) as wpool, \
             tc.tile_pool(name="nhpool", bufs=1) as nhpool, \
             tc.tile_pool(name="opool", bufs=8) as opool, \
             tc.tile_pool(name="bpool", bufs=1) as bpool, \
             tc.tile_pool(name="ps", bufs=8, space="PSUM") as ps:

            # PE warm-up: the HAM clock gate holds the PE at 1.2 GHz until
            # it has been busy ~3.4 µs. Dummy matmuls (no data deps beyond
            # one memset) keep the PE active while the first input tiles
            # stream in, so real matmuls start near 2.4 GHz. The memset
            # rides GpSimd, whose queue opens right after the preamble.
            warm_sb = bpool.tile([P, P], F16)
            nc.gpsimd.memset(warm_sb[:], 0.0)

            b1_sb = bpool.tile([P, HC], F32)
            # Tiny primer transfers lead both HWDGE rings: if the first-
            # transfer cold cost is ring-spin-up (not per-transfer), the
            # first real weight/c tiles then run at warm rate.
            nc.sync.dma_start(b1_sb[:, 0:1], b1[:, 0:1])
            nc.scalar.dma_start(b1_sb[:, 1:2], b1[:, 1:2])
            # The rest of b_ih isn't needed until the first group drains
            # (~65 µs); keep it off the HWDGE rings (SWDGE via GpSimd).
            nc.gpsimd.dma_start(b1_sb[:, 2:], b1[:, 2:])

            c_sb = cpool.tile([P, KO16, BC], F16)
            c8_sb = cpool.tile([P, 2 * KP8, BC], F8)
            nh_sb = nhpool.tile([P, HC, BC], F16)

            # Stores are deferred one group: group g's stores are emitted
            # after group g+1's loads, so when the sync sequencer reaches
            # them the producing compute finished long ago and the ring
            # never head-of-line blocks on a store waiting for compute.
            deferred = []

            def flush_deferred():
                for fn in deferred:
                    fn()
                deferred.clear()

            # mm1: nh^T[h, b] = tanh(W_ih @ combined^T + b_ih)
            # G-sized PSUM groups ping-pong across the 8 banks: while one
            # group's banks drain through ACT, the next group accumulates
            # — group boundaries cost the PE almost nothing.
            for g in range(HC // G):
                psums = [ps.tile([P, BC], F32, tag="ps", name=f"ps{i}")
                         for i in range(G)]
                if g == 0:
                    # The first c and w1 half-tiles land ~2.5 µs after the
                    # rings open; the warm matmuls bridge until then and
                    # start the ~3.4 µs HAM ramp.
                    for _ in range(NWARM):
                        nc.tensor.matmul(
                            psums[G - 1][:, :P], lhsT=warm_sb[:],
                            rhs=warm_sb[:],
                            start=True, stop=True, skip_group_check=True,
                        )
                h0 = g * G
                for ko0 in range(0, KO16, 2):
                    if g == 0:
                        # c rides the ACT HWDGE ring: descriptor pushes for
                        # the first c and w1 tiles then run in parallel on
                        # two queues, and during all of group 0 the sync
                        # ring carries only weights.
                        nc.scalar.dma_start(c_sb[:, ko0:ko0 + 2], c[:, ko0:ko0 + 2])
                        if ko0 == 2:
                            # Preload the ACT tanh table set (~1.3 µs)
                            # during the ramp, not at the first drain.
                            act_warm = bpool.tile([1, 1], F32)
                            nc.scalar.activation(
                                act_warm[:], warm_sb[:1, :1], AF.Tanh)
                        if ko0 == 4:
                            # fp8 tail of c: tiny, needed only at the end
                            # of the group — push during the ramp.
                            nc.scalar.dma_start(c8_sb[:], c8[:])
                    w1_sb = wpool.tile([P, 2, G, P], F16, tag="w")
                    if g == 0 and ko0 <= 2:
                        # The first two weight tiles are split into two
                        # half pushes so the first matmuls can start after
                        # ~256 KiB instead of ~512 KiB of ring traffic.
                        nc.sync.dma_start(
                            w1_sb[:, :, :4], w1[:, ko0:ko0 + 2, h0:h0 + 4])
                        nc.sync.dma_start(
                            w1_sb[:, :, 4:], w1[:, ko0:ko0 + 2, h0 + 4:h0 + 8])
                    else:
                        nc.sync.dma_start(
                            w1_sb[:], w1[:, ko0:ko0 + 2, h0:h0 + G])
                    for kk in range(2):
                        for i in range(G):
                            nc.tensor.matmul(
                                psums[i][:],
                                lhsT=w1_sb[:, kk, i],
                                rhs=c_sb[:, ko0 + kk],
                                start=(ko0 + kk == 0),
                                stop=False,
                            )
                # fp8 DoubleRow tail: each instruction contracts 256 k
                # (2 paired k-tiles) at the same 512-cycle cost as one
                # fp16 matmul — 2x FLOPs/instruction. Both the fp16 and
                # fp8 partials carry the same x64 weight pre-scale, so
                # they accumulate into the SAME PSUM bank; ACT's
                # scale=1/64 undoes it at eviction.
                for kp in range(KP8):
                    w18_sb = wpool.tile([P, 2, G, P], F8, tag="w", name="w18")
                    nc.sync.dma_start(w18_sb[:], w18[:, kp, :, h0:h0 + G])
                    for i in range(G):
                        nc.tensor.matmul(
                            psums[i][:],
                            lhsT=w18_sb[:, :, i],
                            rhs=c8_sb[:, 2 * kp:2 * kp + 2],
                            start=False,
                            stop=(kp == KP8 - 1),
                            perf_mode=DR,
                        )
                flush_deferred()
                for i in range(G):
                    hc = g * G + i
                    nc.scalar.activation(
                        nh_sb[:, hc], psums[i][:], AF.Tanh,
                        bias=b1_sb[:, hc:hc + 1], scale=1.0 / SW,
                    )
                    # nhT stores ride SWDGE: no HWDGE ring time spent.
                    nc.gpsimd.dma_start(
                        nhT[hc * P:(hc + 1) * P, :], nh_sb[:, hc])

            # mm2: out^T[o, b] = W_ho @ nh^T + b_ho
            # Groups of [8, 4, 2, 2] o-chunks: trailing groups ping-pong
            # through the 8 PSUM banks (no boundary stall) and shrink so
            # the post-last-matmul drain chain is as short as possible.
            for g0, gsz in ((0, 8), (8, 4), (12, 2), (14, 2)):
                psums = [ps.tile([P, BC], F32, tag="ps", name=f"ps{i}")
                         for i in range(gsz)]
                for ho0 in range(0, HC, 2):
                    w2_sb = wpool.tile(
                        [P, 2, G, P], F16, tag="w", name="w2_sb")[:, :, :gsz]
                    nc.sync.dma_start(
                        w2_sb[:], w2[:, ho0:ho0 + 2, g0:g0 + gsz])
                    for kk in range(2):
                        for i in range(gsz):
                            nc.tensor.matmul(
                                psums[i][:],
                                lhsT=w2_sb[:, kk, i],
                                rhs=nh_sb[:, ho0 + kk],
                                start=(ho0 + kk == 0),
                                stop=(ho0 + kk == HC - 1),
                            )
                flush_deferred()
                # Evict PSUM through both DVE and ACT in parallel (raw
                # copies; b_ho is added on the host). ACT-evicted tiles
                # store via the ACT HWDGE ring right behind their copy;
                # DVE-evicted tiles store via the sync ring, deferred one
                # group so the ring never waits on the copy.
                last = (g0 + gsz == OC)
                for i in range(gsz):
                    oc = g0 + i
                    o_sb = opool.tile([P, BC], F16, tag="osb")
                    if i % 2:
                        nc.scalar.activation(o_sb[:], psums[i][:], AF.Copy)
                        nc.scalar.dma_start(
                            outT[oc * P:(oc + 1) * P, :], o_sb[:])
                    else:
                        nc.vector.tensor_copy(o_sb[:], psums[i][:])
                        st = (lambda oc=oc, o_sb=o_sb: nc.sync.dma_start(
                            outT[oc * P:(oc + 1) * P, :], o_sb[:]))
                        if last:
                            st()      # no deferral on the final group
                        else:
                            deferred.append(st)
            flush_deferred()

    nc.compile()
    return nc


def _shard_inputs(x, hidden, W_ih, b_ih, W_ho, b_ho):
    combined = np.concatenate([x, hidden], axis=1)  # [B, K1]
    K16 = KO16 * P                                  # fp16 k-range (3584)
    W1s = W_ih.astype(np.float32) * SW
    w1L = np.ascontiguousarray(
        W1s[:, :K16].reshape(HC, P, KO16, P).transpose(3, 2, 0, 1)
    ).astype(NPF16)  # [ki, ko, hc, h]
    w18L = np.ascontiguousarray(
        np.clip(W1s[:, K16:], -240, 240).astype(E4)
        .reshape(HC, P, KP8, 2, P).transpose(4, 2, 3, 0, 1)
    )  # [ki, kp, kk, hc, h]
    w2L = np.ascontiguousarray(
        W_ho.reshape(OC, P, HC, P).transpose(3, 2, 0, 1)
    ).astype(NPF16)  # [hi, ho, oc, o]
    b1L = np.ascontiguousarray(b_ih.reshape(HC, P).T)
    in_maps = []
    for cix in range(NCORES):
        cc = combined[cix * BC:(cix + 1) * BC]  # [BC, K1]
        cL = np.ascontiguousarray(
            cc[:, :K16].reshape(BC, KO16, P).transpose(2, 1, 0)).astype(NPF16)
        c8L = np.ascontiguousarray(
            np.clip(cc[:, K16:], -240, 240).astype(E4)
            .reshape(BC, 2 * KP8, P).transpose(2, 1, 0))
        in_maps.append(
            {"c": cL, "c8": c8L, "w1": w1L, "w18": w18L,
             "b1": b1L, "w2": w2L}
        )
    return in_maps


def _run(in_maps, **kwargs):
    nc = _build()
    return bass_utils.run_bass_kernel_spmd(
        nc, in_maps, core_ids=list(range(NCORES)), **kwargs
    )


def kernel(x, hidden, W_ih, b_ih, W_ho, b_ho):
    x = np.asarray(x, dtype=np.float32)
    hidden = np.asarray(hidden, dtype=np.float32)
    W_ih = np.asarray(W_ih, dtype=np.float32)
    b_ih = np.asarray(b_ih, dtype=np.float32)
    W_ho = np.asarray(W_ho, dtype=np.float32)
    b_ho = np.asarray(b_ho, dtype=np.float32)

    in_maps = _shard_inputs(x, hidden, W_ih, b_ih, W_ho, b_ho)
    res = _run(in_maps)
    output = np.concatenate(
        [r["outT"].T.astype(np.float32) for r in res.results], axis=0) + b_ho
    new_hidden = np.concatenate(
        [r["nhT"].T for r in res.results], axis=0).astype(np.float32)
    return output, new_hidden
